# revision 3
# baseline (speedup 1.0000x reference)
"""BiGCN (graphcl) Trainium2 kernel — 8-core SPMD, fp8 DoubleRow edition.

Decomposition (per branch, A = sym-normalized adjacency with self loops):
    h1     = relu(A @ (xv @ W1) + b1)
    pooled = M @ h1 @ W2 + (c_g + 1) * b2        with M = T @ A (dense [B, nv])
    h      = [bu | td];  out = relu(h @ p_w1 + p_b1) @ p_w2 + p_b2

Sharding: 128-node tiles are assigned to (core, position) by a balanced
snake deal over per-tile edge-chunk counts, so the shared SPMD program's
per-position chunk count F[pos] (maxed over cores) wastes almost nothing.

Per tile, layer 1 splits into
  self-loop part: one fp8 DoubleRow matmul from a dense preloaded block
      xselfT[feat, node] * dinv2  ->  psum_h1 (start)
  edge part: host-staged per-core edge streams (gathered source rows
      pre-multiplied by norm, fp8, partition-major so every DMA reads
      >=2KB contiguous per partition). Per chunk:
      x_chunk fp8 DoubleRow (K=256 in one op) -> psum_A -> cast fp8
      psum_h1 += Q^T @ xw  (one-hot Q streamed in the same blob)
Layer 2 + pooling collapse into G += [h1 pair]^T (x) [M pair] fp8
DoubleRow pairs accumulated in one PSUM bank; G is the only collective
(64KB AllReduce), then the tiny MLP head runs replicated on every core.

fp8 scale trick: W1, b1 are staged x16 (relu is positively homogeneous),
W2 staged /16 — keeps W1 out of fp8's subnormal range.
"""
import numpy as np

N_CORES = 8
N = 100000
NV = N + 1
S = 12544                 # nodes per core = 98 * 128
T_TILES = S // 128        # 98
G_TILES = N_CORES * T_TILES   # 784 global tiles
NVP = N_CORES * S
B = 128
IN = 256
HID = 128

W1_SCALE = 16.0
DEBUG = False


# ----------------------------------------------------------------- host prep
def _build_branch(s_e, d_e, batch):
    """s_e/d_e: edge endpoints WITHOUT self loops (E real + B virtual edges).

    Returns per-branch staging data. Self loops enter deg, M and the dense
    per-tile self blocks, not the edge stream.
    """
    deg = np.bincount(d_e, minlength=NV).astype(np.float64) + 1.0
    dinv = 1.0 / np.sqrt(deg)
    enorm = (dinv[s_e] * dinv[d_e]).astype(np.float64)

    # M = T @ A over full A (edges + self loops)
    es = np.concatenate([s_e, np.arange(NV, dtype=np.int64)])
    ed = np.concatenate([d_e, np.arange(NV, dtype=np.int64)])
    en = np.concatenate([enorm, 1.0 / deg])
    M = np.zeros((B, NVP), dtype=np.float64)
    real = ed < N
    np.add.at(M, (batch[ed[real]].astype(np.int64), es[real]), en[real])
    virt = ~real
    if virt.any():
        M += np.bincount(es[virt], weights=en[virt], minlength=NVP)[None, :]

    # balanced tile -> (core, pos) assignment over edge-chunk counts
    gt = d_e // 128
    counts = np.bincount(gt, minlength=G_TILES)
    ct = -(-counts // 128)                      # ceil, 0 for empty tiles
    order = np.argsort(-ct, kind="stable")      # tiles by count desc
    tile_at = np.empty((N_CORES, T_TILES), dtype=np.int64)
    for r, tl in enumerate(order):
        row, idx = divmod(r, N_CORES)
        core = idx if (row % 2 == 0) else N_CORES - 1 - idx
        tile_at[core, row] = tl
    F = ct[order[::N_CORES]].astype(np.int64)   # per-position max over cores
    C = int(F.sum())
    chunk_base = np.concatenate([[0], np.cumsum(F)])

    # group edge entries by global tile
    eorder = np.argsort(gt, kind="stable")
    es_s, ed_s, en_s = s_e[eorder], d_e[eorder], enorm[eorder]
    tile_starts = np.concatenate([[0], np.cumsum(counts)])

    ent_src = np.zeros((N_CORES, C * 128), dtype=np.int64)
    ent_norm = np.zeros((N_CORES, C * 128), dtype=np.float32)
    ent_slot = np.zeros((N_CORES, C * 128), dtype=np.int64)
    for k in range(N_CORES):
        for t in range(T_TILES):
            tl = tile_at[k, t]
            a, bnd = tile_starts[tl], tile_starts[tl + 1]
            if bnd == a:
                continue
            off = chunk_base[t] * 128
            m = bnd - a
            ent_src[k, off:off + m] = es_s[a:bnd]
            ent_norm[k, off:off + m] = en_s[a:bnd]
            ent_slot[k, off:off + m] = ed_s[a:bnd] - tl * 128
    return dict(ent_src=ent_src, ent_norm=ent_norm, ent_slot=ent_slot,
                F=F, C=C, M=M, tile_at=tile_at, dinv2=(1.0 / deg))


def _host_prep(x, emb_w, edge_index, batch):
    xv = np.concatenate([np.asarray(x, np.float32),
                         np.asarray(emb_w, np.float32)], axis=0)
    roots = np.searchsorted(batch, np.arange(B, dtype=batch.dtype)).astype(np.int64)
    ei0 = edge_index[0].astype(np.int64)
    ei1 = edge_index[1].astype(np.int64)
    vs = np.full(B, N, dtype=np.int64)
    br = {
        "td": _build_branch(np.concatenate([ei0, vs]), np.concatenate([ei1, roots]), batch),
        "bu": _build_branch(np.concatenate([ei1, roots]), np.concatenate([ei0, vs]), batch),
    }
    counts_g = np.bincount(batch, minlength=B).astype(np.float64)
    return xv, br, counts_g


# ------------------------------------------------------- walrus wait limiter
def _split_excess_waits(nc, limit=1):
    import concourse.mybir as mybir
    n_added = 0
    for bb in nc.main_func.blocks:
        insts = bb.instructions
        new_list = []
        for inst in insts:
            si = inst.sync_info
            if si is not None and si.on_wait and len(si.on_wait) > limit:
                waits = list(si.on_wait)
                extra, keep = waits[:-limit], waits[-limit:]
                for w in extra:
                    noop = mybir.InstNoOp(name=f"I-wsplit-{nc.next_id()}", ins=[], outs=[])
                    noop.engine = inst.engine
                    noop.sync_info = mybir.SyncInfo(on_wait=[w], on_update=[])
                    nc.register_instruction(noop, overwrite=True)
                    new_list.append(noop)
                    n_added += 1
                inst.sync_info = mybir.SyncInfo(on_wait=keep, on_update=list(si.on_update or []))
            new_list.append(inst)
        insts[:] = new_list
    return n_added


# ------------------------------------------------------------ device program
def _build_program(F_td, F_bu):
    import concourse.bass as bass
    import concourse.mybir as mybir
    import concourse.tile as tile

    f32 = mybir.dt.float32
    fp8 = mybir.dt.float8e4
    DR = mybir.MatmulPerfMode.DoubleRow

    nc = bass.Bass(target_bir_lowering=False, trn_type="TRN2", num_swdge_queues=4)

    dram_in = {}
    for bn, C in (("td", int(F_td.sum())), ("bu", int(F_bu.sum()))):
        dram_in[f"xs_{bn}"] = nc.dram_tensor(f"xs_{bn}", [128, C * 3, 128], fp8, kind="ExternalInput")
        dram_in[f"xself_{bn}"] = nc.dram_tensor(f"xself_{bn}", [128, T_TILES * 2, 128], fp8, kind="ExternalInput")
        dram_in[f"mt_{bn}"] = nc.dram_tensor(f"mt_{bn}", [128, T_TILES, 128], fp8, kind="ExternalInput")
        dram_in[f"w1_{bn}"] = nc.dram_tensor(f"w1_{bn}", [128, 2, HID], fp8, kind="ExternalInput")
        dram_in[f"b1b_{bn}"] = nc.dram_tensor(f"b1b_{bn}", [128, 4, HID], f32, kind="ExternalInput")
        dram_in[f"w2_{bn}"] = nc.dram_tensor(f"w2_{bn}", [HID, HID], f32, kind="ExternalInput")
        dram_in[f"pb_{bn}"] = nc.dram_tensor(f"pb_{bn}", [HID, B], f32, kind="ExternalInput")
    dram_in["pw1"] = nc.dram_tensor("pw1", [2 * HID, 2 * HID], f32, kind="ExternalInput")
    dram_in["pb1"] = nc.dram_tensor("pb1", [128, 2], f32, kind="ExternalInput")
    dram_in["pw2"] = nc.dram_tensor("pw2", [2 * HID, HID], f32, kind="ExternalInput")
    dram_in["pb2"] = nc.dram_tensor("pb2", [128, 1], f32, kind="ExternalInput")
    out_t = nc.dram_tensor("outT", [HID, B], f32, kind="ExternalOutput")

    with tile.TileContext(nc) as tc:
        with (
            tc.tile_pool(name="const", bufs=1) as cpool,
            tc.tile_pool(name="stream", bufs=6) as spool,
            tc.tile_pool(name="work", bufs=8) as wpool,
            tc.tile_pool(name="psA", bufs=3, space="PSUM") as psA,
            tc.tile_pool(name="psH", bufs=3, space="PSUM") as psH,
            tc.tile_pool(name="psG", bufs=2, space="PSUM") as psG,
            tc.tile_pool(name="dram", bufs=1, space="DRAM") as dpool,
        ):
            dma_engines = [nc.sync, nc.scalar, nc.gpsimd]
            dma_rr = [0]

            # ---- per-branch constants: preload everything up front -------
            consts = {}
            for i, (bn, F) in enumerate((("td", F_td), ("bu", F_bu))):
                w1sb = cpool.tile([128, 2, HID], fp8, name=f"w1sb_{bn}")
                nc.sync.dma_start(w1sb[:], dram_in[f"w1_{bn}"][:, :, :])
                b1b4 = cpool.tile([128, 4, HID], f32, name=f"b1b4_{bn}")
                nc.scalar.dma_start(b1b4[:], dram_in[f"b1b_{bn}"][:, :, :])
                xself = cpool.tile([128, T_TILES * 2, 128], fp8, name=f"xself_{bn}")
                nc.gpsimd.dma_start(xself[:], dram_in[f"xself_{bn}"][:, :, :])
                mtsb = cpool.tile([128, T_TILES, 128], fp8, name=f"mtsb_{bn}")
                nc.sync.dma_start(mtsb[:], dram_in[f"mt_{bn}"][:, :, :])
                consts[bn] = (w1sb, b1b4, xself, mtsb)

            ar_out = {}
            for bn, F in (("td", F_td), ("bu", F_bu)):
                C = int(F.sum())
                xs = dram_in[f"xs_{bn}"]
                w1sb, b1b4, xself, mtsb = consts[bn]

                psum_G = psG.tile([HID, B], f32, name=f"psum_G_{bn}", tag="G")

                xws_grp = None
                psum_A = None

                def emit_qmms(lst):
                    for (qq, xg, cc2, ph, tt2, sp) in lst:
                        nc.tensor.matmul(ph[:, tt2, :], qq, xg[:, cc2, :],
                                         start=False, stop=sp)

                def flush_pend(nslices, drain=False):
                    # cast current group, emit PREVIOUS group's Q-matmuls
                    # (1-group software pipeline so PE never waits on the
                    # fresh cast)
                    if pend:
                        nc.vector.tensor_copy(xws_grp[:, 0:nslices, :],
                                              psum_A[:, 0:nslices, :])
                        emit_qmms(pend_prev)
                        pend_prev[:] = list(pend)
                        pend.clear()
                    if drain:
                        emit_qmms(pend_prev)
                        pend_prev.clear()

                c = 0
                h1_grp = None
                psum_h1 = None
                xt2 = None
                pend: list = []
                pend_prev: list = []
                pend_m: list = []
                for t in range(T_TILES):
                    tt = t % 4
                    if tt == 0:
                        psum_h1 = psH.tile([128, 4, HID], f32, name="psum_h1", tag="H")
                        h1_grp = wpool.tile([128, 4, HID], fp8, name="h1_grp")
                    ft = int(F[t])
                    # self-loop part: one DoubleRow, first write of this tile's
                    # psum slice. start=True only on the group's first matmul:
                    # start marks the WHOLE 2KB psum bank pending-zero, so a
                    # per-tile start would wipe earlier tiles whose pipelined
                    # Q-matmuls land later. Pending-zero propagation makes the
                    # first write to each slice an overwrite either way.
                    nc.tensor.matmul(psum_h1[:, tt, :], xself[:, 2 * t:2 * t + 2, :],
                                     w1sb[:, :, :], start=(tt == 0), stop=(ft == 0),
                                     perf_mode=DR)
                    for j in range(ft):
                        cc = c % 4
                        if cc == 0:
                            psum_A = psA.tile([128, 4, HID], f32, name="psum_A", tag="A")
                            xws_grp = wpool.tile([128, 4, HID], fp8, name="xws_grp")
                        if c % 8 == 0:
                            nld = min(8, C - c)
                            xt2 = spool.tile([128, 24, 128], fp8, name="xt2")
                            eng = dma_engines[dma_rr[0] % len(dma_engines)]
                            dma_rr[0] += 1
                            eng.dma_start(xt2[:, 0:nld * 3, :],
                                          xs[:, c * 3:(c + nld) * 3, :])
                        sl = (c % 8) * 3
                        nc.tensor.matmul(psum_A[:, cc, :], xt2[:, sl:sl + 2, :],
                                         w1sb[:, :, :], start=True, stop=True,
                                         perf_mode=DR)
                        pend.append((xt2[:, sl + 2, :], xws_grp, cc, psum_h1, tt,
                                     j == ft - 1))
                        if cc == 3 or c == C - 1:
                            flush_pend(cc + 1)
                        c += 1
                    if tt == 3 or t == T_TILES - 1:
                        flush_pend(((c - 1) % 4) + 1 if pend else 0, drain=True)
                        ns = tt + 1
                        tmp = wpool.tile([128, 4, HID], f32, name="h1tmp")
                        nc.vector.tensor_tensor(tmp[:, 0:ns, :], psum_h1[:, 0:ns, :],
                                                b1b4[:, 0:ns, :], op=mybir.AluOpType.add)
                        nc.scalar.activation(h1_grp[:, 0:ns, :], tmp[:, 0:ns, :],
                                             mybir.ActivationFunctionType.Relu)
                        t0 = t - tt
                        # G += h1-pair (x) M-pair, one DoubleRow per 2 tiles,
                        # one 4-tile group delayed so PE never waits on relu
                        for (h1p, pt0, pns) in pend_m:
                            for jj in range(0, pns, 2):
                                nc.tensor.matmul(
                                    psum_G[:], h1p[:, jj:jj + 2, :],
                                    mtsb[:, pt0 + jj:pt0 + jj + 2, :],
                                    start=(pt0 + jj == 0), stop=False,
                                    perf_mode=DR)
                        pend_m = [(h1_grp, t0, ns)]
                        if t == T_TILES - 1:
                            for (h1p, pt0, pns) in pend_m:
                                for jj in range(0, pns, 2):
                                    nc.tensor.matmul(
                                        psum_G[:], h1p[:, jj:jj + 2, :],
                                        mtsb[:, pt0 + jj:pt0 + jj + 2, :],
                                        start=(pt0 + jj == 0),
                                        stop=(pt0 + jj + 2 >= T_TILES),
                                        perf_mode=DR)
                            pend_m = []
                g = cpool.tile([HID, B], f32, name=f"g_{bn}")
                nc.vector.tensor_copy(g[:], psum_G[:])
                # per-branch AllReduce: td's runs while bu branch computes
                arin = dpool.tile([HID, B], f32, name=f"arin_{bn}")
                arout = dpool.tile([HID, B], f32, addr_space="Shared", name=f"arout_{bn}")
                nc.gpsimd.dma_start(arin[:], g[:])
                nc.gpsimd.collective_compute(
                    "AllReduce", mybir.AluOpType.add,
                    replica_groups=[list(range(N_CORES))],
                    ins=[arin[:]], outs=[arout[:]],
                )
                ar_out[bn] = arout

            # ---- MLP head (replicated on every core, transposed layout) ----
            pw1 = cpool.tile([128, 2, 2 * HID], f32)
            nc.gpsimd.dma_start(pw1[:], dram_in["pw1"].rearrange("(kc p) n -> p kc n", p=128))
            pb1 = cpool.tile([128, 2], f32)
            nc.gpsimd.dma_start(pb1[:], dram_in["pb1"][:, :])
            pw2 = cpool.tile([128, 2, HID], f32)
            nc.gpsimd.dma_start(pw2[:], dram_in["pw2"].rearrange("(kc p) n -> p kc n", p=128))
            pb2 = cpool.tile([128, 1], f32)
            nc.gpsimd.dma_start(pb2[:], dram_in["pb2"][:, :])

            pool_t = {}
            for i, bn in enumerate(("td", "bu")):
                garr = cpool.tile([HID, B], f32, name=f"garr_{bn}")
                nc.gpsimd.dma_start(garr[:], ar_out[bn][:])
                w2sb = cpool.tile([HID, HID], f32, name=f"w2sb_{bn}")
                nc.gpsimd.dma_start(w2sb[:], dram_in[f"w2_{bn}"][:, :])
                pbsb = cpool.tile([HID, B], f32, name=f"pbsb_{bn}")
                nc.gpsimd.dma_start(pbsb[:], dram_in[f"pb_{bn}"][:, :])

                ps_p = psA.tile([HID, B], f32, name="ps_p", tag="A")
                nc.tensor.matmul(ps_p[:], w2sb[:], garr[:], start=True, stop=True)
                pt = cpool.tile([HID, B], f32, name=f"pool_{bn}")
                nc.vector.tensor_tensor(pt[:], ps_p[:], pbsb[:], op=mybir.AluOpType.add)
                pool_t[bn] = pt                                      # pooled^T [f, g]

            r1 = []
            for hh in range(2):
                ps1 = psA.tile([128, B], f32, name="ps1", tag="A")
                nc.tensor.matmul(ps1[:], pw1[:, 0, hh * 128:(hh + 1) * 128],
                                 pool_t["bu"][:], start=True, stop=False)
                nc.tensor.matmul(ps1[:], pw1[:, 1, hh * 128:(hh + 1) * 128],
                                 pool_t["td"][:], start=False, stop=True)
                r = wpool.tile([128, B], f32, name=f"r1_{hh}")
                nc.scalar.activation(r[:], ps1[:], mybir.ActivationFunctionType.Relu,
                                     bias=pb1[:, hh:hh + 1])
                r1.append(r)
            ps2 = psH.tile([HID, B], f32, name="ps2", tag="H")
            for hh in range(2):
                nc.tensor.matmul(ps2[:], pw2[:, hh, :], r1[hh][:],
                                 start=(hh == 0), stop=(hh == 1))
            ofin = wpool.tile([HID, B], f32, name="ofin")
            nc.vector.tensor_scalar(ofin[:], ps2[:], pb2[:, 0:1], None,
                                    op0=mybir.AluOpType.add)
            nc.gpsimd.dma_start(out_t[:, :], ofin[:])

    _split_excess_waits(nc, limit=1)
    return nc


# ------------------------------------------------------------------- staging
def _stage_core(k, xvp_f32, br, counts_g, inputs, np_fp8):
    m = {}
    for bn in ("td", "bu"):
        d = br[bn]
        C = d["C"]
        tiles = d["tile_at"][k]                         # [98] global tile ids

        # edge stream blob [128, C*3, 128]: per chunk 3 slices (x-k0, x-k1, Q)
        src = d["ent_src"][k]
        nrm = d["ent_norm"][k]
        xg = xvp_f32[src] * nrm[:, None]                # [C*128, 256] f32
        xpart = xg.reshape(C, 128, IN).transpose(2, 0, 1)   # [256, C, 128]
        xpart = xpart.reshape(2, 128, C, 128).transpose(1, 2, 0, 3)  # [128,C,2,128]
        slot = d["ent_slot"][k]
        Q = np.zeros((C, 128, 128), dtype=np.float32)
        Q.reshape(C * 128, 128)[np.arange(C * 128), slot] = 1.0
        qpart = Q.transpose(1, 0, 2)                    # [128, C, 128]
        blob = np.concatenate([xpart, qpart[:, :, None, :]], axis=2)  # [128,C,3,128]
        m[f"xs_{bn}"] = np.ascontiguousarray(
            blob.reshape(128, C * 3, 128), dtype=np_fp8)

        # dense self block [128, 98*2, 128]: xselfT[feat, tile, kc, node]
        nodes = (tiles[:, None] * 128 + np.arange(128)[None, :]).reshape(-1)
        xs_blk = xvp_f32[nodes] * d["dinv2"][np.minimum(nodes, NV - 1)][:, None]
        xs_blk[nodes >= NV] = 0.0
        A = xs_blk.reshape(T_TILES, 128, IN).transpose(2, 0, 1)  # [256, 98, 128]
        A = A.reshape(2, 128, T_TILES, 128).transpose(1, 2, 0, 3)  # [128,98,2,128]
        m[f"xself_{bn}"] = np.ascontiguousarray(
            A.reshape(128, T_TILES * 2, 128), dtype=np_fp8)

        # M^T columns for this core's tiles, [128, 98, 128] = [node, tile, graph]
        Mc = d["M"][:, nodes].reshape(B, T_TILES, 128)  # [g, t, p]
        m[f"mt_{bn}"] = np.ascontiguousarray(Mc.transpose(2, 1, 0), dtype=np_fp8)

        w1 = np.asarray(inputs[f"{bn}_w1"], np.float32) * W1_SCALE
        m[f"w1_{bn}"] = np.ascontiguousarray(
            w1.reshape(2, 128, HID).transpose(1, 0, 2), dtype=np_fp8)
        b1b = np.broadcast_to(
            np.asarray(inputs[f"{bn}_b1"], np.float32) * W1_SCALE, (128, 4, HID))
        m[f"b1b_{bn}"] = np.ascontiguousarray(b1b, dtype=np.float32)
        m[f"w2_{bn}"] = np.ascontiguousarray(
            np.asarray(inputs[f"{bn}_w2"], np.float32) / W1_SCALE)
        m[f"pb_{bn}"] = np.ascontiguousarray(
            np.outer(np.asarray(inputs[f"{bn}_b2"], np.float64), counts_g + 1.0),
            dtype=np.float32)
    m["pw1"] = np.ascontiguousarray(np.asarray(inputs["p_w1"], np.float32))
    m["pb1"] = np.ascontiguousarray(
        np.asarray(inputs["p_b1"], np.float32).reshape(2, 128).T)
    m["pw2"] = np.ascontiguousarray(np.asarray(inputs["p_w2"], np.float32))
    m["pb2"] = np.asarray(inputs["p_b2"], np.float32).reshape(128, 1).copy()
    return m


def _run(inputs, trace=False):
    import ml_dtypes
    from concourse import bass_utils

    x = np.asarray(inputs["x"])
    edge_index = np.asarray(inputs["edge_index"])
    batch = np.asarray(inputs["batch"])
    xv, br, counts_g = _host_prep(x, inputs["emb_w"], edge_index, batch)
    xvp = np.zeros((NVP, IN), dtype=np.float32)
    xvp[:NV] = xv

    np_fp8 = ml_dtypes.float8_e4m3
    in_maps = [_stage_core(k, xvp, br, counts_g, inputs, np_fp8)
               for k in range(N_CORES)]
    nc = _build_program(br["td"]["F"], br["bu"]["F"])
    last = None
    for attempt in range(3):
        try:
            res = bass_utils.run_bass_kernel_spmd(
                nc, in_maps, core_ids=list(range(N_CORES)), trace=trace)
            break
        except Exception as e:   # transient NRT device errors recover on retry
            last = e
    else:
        raise last
    out = np.ascontiguousarray(res.results[0]["outT"].T, dtype=np.float32)
    return out, res


def kernel(**inputs) -> np.ndarray:
    out, _ = _run(inputs, trace=False)
    return out


# revision 10
# speedup vs baseline: 1.0078x; 1.0078x over previous
"""BiGCN (graphcl) Trainium2 kernel — 8-core SPMD, fp8 DoubleRow edition.

Decomposition (per branch, A = sym-normalized adjacency with self loops):
    h1     = relu(A @ (xv @ W1) + b1)
    pooled = M @ h1 @ W2 + (c_g + 1) * b2        with M = T @ A (dense [B, nv])
    h      = [bu | td];  out = relu(h @ p_w1 + p_b1) @ p_w2 + p_b2

Sharding: 128-node tiles are assigned to (core, position) by a balanced
snake deal over per-tile edge-chunk counts, so the shared SPMD program's
per-position chunk count F[pos] (maxed over cores) wastes almost nothing.

Per tile, layer 1 splits into
  self-loop part: one fp8 DoubleRow matmul from a dense preloaded block
      xselfT[feat, node] * dinv2  ->  psum_h1 (start)
  edge part: host-staged per-core edge streams (gathered source rows
      pre-multiplied by norm, fp8, partition-major so every DMA reads
      >=2KB contiguous per partition). Per chunk:
      x_chunk fp8 DoubleRow (K=256 in one op) -> psum_A -> cast fp8
      psum_h1 += Q^T @ xw  (one-hot Q streamed in the same blob)
Layer 2 + pooling collapse into G += [h1 pair]^T (x) [M pair] fp8
DoubleRow pairs accumulated in one PSUM bank; G is the only collective
(64KB AllReduce), then the tiny MLP head runs replicated on every core.

fp8 scale trick: W1, b1 are staged x16 (relu is positively homogeneous),
W2 staged /16 — keeps W1 out of fp8's subnormal range.
"""
import numpy as np

N_CORES = 8
N = 100000
NV = N + 1
S = 12544                 # nodes per core = 98 * 128
T_TILES = S // 128        # 98
G_TILES = N_CORES * T_TILES   # 784 global tiles
NVP = N_CORES * S
B = 128
IN = 256
HID = 128

W1_SCALE = 16.0
DEBUG = False


# ----------------------------------------------------------------- host prep
def _build_branch(s_e, d_e, batch):
    """s_e/d_e: edge endpoints WITHOUT self loops (E real + B virtual edges).

    Returns per-branch staging data. Self loops enter deg, M and the dense
    per-tile self blocks, not the edge stream.
    """
    deg = np.bincount(d_e, minlength=NV).astype(np.float64) + 1.0
    dinv = 1.0 / np.sqrt(deg)
    enorm = (dinv[s_e] * dinv[d_e]).astype(np.float64)

    # M = T @ A over full A (edges + self loops)
    es = np.concatenate([s_e, np.arange(NV, dtype=np.int64)])
    ed = np.concatenate([d_e, np.arange(NV, dtype=np.int64)])
    en = np.concatenate([enorm, 1.0 / deg])
    M = np.zeros((B, NVP), dtype=np.float64)
    real = ed < N
    np.add.at(M, (batch[ed[real]].astype(np.int64), es[real]), en[real])
    virt = ~real
    if virt.any():
        M += np.bincount(es[virt], weights=en[virt], minlength=NVP)[None, :]

    # balanced tile -> (core, pos) assignment over edge-chunk counts
    gt = d_e // 128
    counts = np.bincount(gt, minlength=G_TILES)
    ct = -(-counts // 128)                      # ceil, 0 for empty tiles
    order = np.argsort(-ct, kind="stable")      # tiles by count desc
    tile_at = np.empty((N_CORES, T_TILES), dtype=np.int64)
    for r, tl in enumerate(order):
        row, idx = divmod(r, N_CORES)
        core = idx if (row % 2 == 0) else N_CORES - 1 - idx
        tile_at[core, row] = tl
    F = ct[order[::N_CORES]].astype(np.int64)   # per-position max over cores
    C = int(F.sum())
    chunk_base = np.concatenate([[0], np.cumsum(F)])

    # group edge entries by global tile
    eorder = np.argsort(gt, kind="stable")
    es_s, ed_s, en_s = s_e[eorder], d_e[eorder], enorm[eorder]
    tile_starts = np.concatenate([[0], np.cumsum(counts)])

    ent_src = np.zeros((N_CORES, C * 128), dtype=np.int64)
    ent_norm = np.zeros((N_CORES, C * 128), dtype=np.float32)
    ent_slot = np.zeros((N_CORES, C * 128), dtype=np.int64)
    for k in range(N_CORES):
        for t in range(T_TILES):
            tl = tile_at[k, t]
            a, bnd = tile_starts[tl], tile_starts[tl + 1]
            if bnd == a:
                continue
            off = chunk_base[t] * 128
            m = bnd - a
            ent_src[k, off:off + m] = es_s[a:bnd]
            ent_norm[k, off:off + m] = en_s[a:bnd]
            ent_slot[k, off:off + m] = ed_s[a:bnd] - tl * 128
    return dict(ent_src=ent_src, ent_norm=ent_norm, ent_slot=ent_slot,
                F=F, C=C, M=M, tile_at=tile_at, dinv2=(1.0 / deg))


def _host_prep(x, emb_w, edge_index, batch):
    xv = np.concatenate([np.asarray(x, np.float32),
                         np.asarray(emb_w, np.float32)], axis=0)
    roots = np.searchsorted(batch, np.arange(B, dtype=batch.dtype)).astype(np.int64)
    ei0 = edge_index[0].astype(np.int64)
    ei1 = edge_index[1].astype(np.int64)
    vs = np.full(B, N, dtype=np.int64)
    br = {
        "td": _build_branch(np.concatenate([ei0, vs]), np.concatenate([ei1, roots]), batch),
        "bu": _build_branch(np.concatenate([ei1, roots]), np.concatenate([ei0, vs]), batch),
    }
    counts_g = np.bincount(batch, minlength=B).astype(np.float64)
    return xv, br, counts_g


# ------------------------------------------------------- walrus wait limiter
def _split_excess_waits(nc, limit=1):
    import concourse.mybir as mybir
    n_added = 0
    for bb in nc.main_func.blocks:
        insts = bb.instructions
        new_list = []
        for inst in insts:
            si = inst.sync_info
            if si is not None and si.on_wait and len(si.on_wait) > limit:
                waits = list(si.on_wait)
                extra, keep = waits[:-limit], waits[-limit:]
                for w in extra:
                    noop = mybir.InstNoOp(name=f"I-wsplit-{nc.next_id()}", ins=[], outs=[])
                    noop.engine = inst.engine
                    noop.sync_info = mybir.SyncInfo(on_wait=[w], on_update=[])
                    nc.register_instruction(noop, overwrite=True)
                    new_list.append(noop)
                    n_added += 1
                inst.sync_info = mybir.SyncInfo(on_wait=keep, on_update=list(si.on_update or []))
            new_list.append(inst)
        insts[:] = new_list
    return n_added


# ------------------------------------------------------------ device program
def _build_program(F_td, F_bu):
    import concourse.bass as bass
    import concourse.mybir as mybir
    import concourse.tile as tile

    f32 = mybir.dt.float32
    fp8 = mybir.dt.float8e4
    DR = mybir.MatmulPerfMode.DoubleRow

    nc = bass.Bass(target_bir_lowering=False, trn_type="TRN2", num_swdge_queues=4)

    dram_in = {}
    for bn, C in (("td", int(F_td.sum())), ("bu", int(F_bu.sum()))):
        dram_in[f"xs_{bn}"] = nc.dram_tensor(f"xs_{bn}", [128, C * 3, 128], fp8, kind="ExternalInput")
        dram_in[f"xself_{bn}"] = nc.dram_tensor(f"xself_{bn}", [128, T_TILES * 2, 128], fp8, kind="ExternalInput")
        dram_in[f"mt_{bn}"] = nc.dram_tensor(f"mt_{bn}", [128, T_TILES, 128], fp8, kind="ExternalInput")
        dram_in[f"w1_{bn}"] = nc.dram_tensor(f"w1_{bn}", [128, 2, HID], fp8, kind="ExternalInput")
        dram_in[f"b1b_{bn}"] = nc.dram_tensor(f"b1b_{bn}", [128, 4, HID], f32, kind="ExternalInput")
        dram_in[f"w2_{bn}"] = nc.dram_tensor(f"w2_{bn}", [HID, HID], f32, kind="ExternalInput")
        dram_in[f"pb_{bn}"] = nc.dram_tensor(f"pb_{bn}", [HID, B], f32, kind="ExternalInput")
    dram_in["pw1"] = nc.dram_tensor("pw1", [2 * HID, 2 * HID], f32, kind="ExternalInput")
    dram_in["pb1"] = nc.dram_tensor("pb1", [128, 2], f32, kind="ExternalInput")
    dram_in["pw2"] = nc.dram_tensor("pw2", [2 * HID, HID], f32, kind="ExternalInput")
    dram_in["pb2"] = nc.dram_tensor("pb2", [128, 1], f32, kind="ExternalInput")
    out_t = nc.dram_tensor("outT", [HID, B], f32, kind="ExternalOutput")

    N_GRP = (T_TILES + 3) // 4

    with tile.TileContext(nc) as tc:
        with (
            tc.tile_pool(name="const", bufs=1) as cpool,
            tc.tile_pool(name="stream", bufs=6) as spool,
            tc.tile_pool(name="selfp", bufs=3) as selfpool,
            tc.tile_pool(name="mtp", bufs=3) as mtpool,
            tc.tile_pool(name="work", bufs=8) as wpool,
            tc.tile_pool(name="psA", bufs=3, space="PSUM") as psA,
            tc.tile_pool(name="psH", bufs=3, space="PSUM") as psH,
            tc.tile_pool(name="psG", bufs=2, space="PSUM") as psG,
            tc.tile_pool(name="dram", bufs=1, space="DRAM") as dpool,
        ):
            dma_engines = [nc.sync, nc.scalar, nc.gpsimd]
            dma_rr = [0]

            def next_eng():
                eng = dma_engines[dma_rr[0] % len(dma_engines)]
                dma_rr[0] += 1
                return eng

            # ---- per-branch small constants -----------------------------
            consts = {}
            for bn in ("td", "bu"):
                w1sb = cpool.tile([128, 2, HID], fp8, name=f"w1sb_{bn}")
                nc.sync.dma_start(w1sb[:], dram_in[f"w1_{bn}"][:, :, :])
                b1b4 = cpool.tile([128, 4, HID], f32, name=f"b1b4_{bn}")
                nc.scalar.dma_start(b1b4[:], dram_in[f"b1b_{bn}"][:, :, :])
                consts[bn] = (w1sb, b1b4)

            gsb = cpool.tile([HID, 2 * B], f32, name="gsb")   # [f, td|bu graphs]
            for bn, F in (("td", F_td), ("bu", F_bu)):
                C = int(F.sum())
                xs = dram_in[f"xs_{bn}"]
                xself_d = dram_in[f"xself_{bn}"]
                mt_d = dram_in[f"mt_{bn}"]
                w1sb, b1b4 = consts[bn]

                # just-in-time per-group xself/mt slices (1 group lookahead)
                xself_t = {}
                mt_t = {}

                def load_group(g):
                    if g >= N_GRP or g in xself_t:
                        return
                    ns_g = min(4, T_TILES - g * 4)
                    xt = selfpool.tile([128, 8, 128], fp8, name="xselfg")
                    next_eng().dma_start(xt[:, 0:2 * ns_g, :],
                                         xself_d[:, g * 8:g * 8 + 2 * ns_g, :])
                    mtt = mtpool.tile([128, 4, 128], fp8, name="mtg")
                    next_eng().dma_start(mtt[:, 0:ns_g, :],
                                         mt_d[:, g * 4:g * 4 + ns_g, :])
                    xself_t[g] = xt
                    mt_t[g] = mtt

                load_group(0)
                load_group(1)

                psum_G = psG.tile([HID, B], f32, name=f"psum_G_{bn}", tag="G")

                xws_grp = None
                psum_A = None

                def emit_qmms(lst):
                    for (qq, xg, cc2, ph, tt2, sp) in lst:
                        nc.tensor.matmul(ph[:, tt2, :], qq, xg[:, cc2, :],
                                         start=False, stop=sp)

                def flush_pend(nslices, drain=False):
                    # cast current group (on the ACT engine — DVE is the
                    # busier one), emit PREVIOUS group's Q-matmuls (1-group
                    # software pipeline so PE never waits on the fresh cast)
                    if pend:
                        nc.scalar.activation(xws_grp[:, 0:nslices, :],
                                             psum_A[:, 0:nslices, :],
                                             mybir.ActivationFunctionType.Copy)
                        emit_qmms(pend_prev)
                        pend_prev[:] = list(pend)
                        pend.clear()
                    if drain:
                        emit_qmms(pend_prev)
                        pend_prev.clear()

                c = 0
                h1_grp = None
                psum_h1 = None
                xt2 = None
                pend: list = []
                pend_prev: list = []
                pend_m: list = []
                for t in range(T_TILES):
                    tt = t % 4
                    gi = t // 4
                    if tt == 0:
                        psum_h1 = psH.tile([128, 4, HID], f32, name="psum_h1", tag="H")
                        h1_grp = wpool.tile([128, 4, HID], fp8, name="h1_grp")
                        load_group(gi + 1)
                    ft = int(F[t])
                    xself = xself_t[gi]
                    # self-loop part: one DoubleRow, first write of this tile's
                    # psum slice. start=True only on the group's first matmul:
                    # start marks the WHOLE 2KB psum bank pending-zero, so a
                    # per-tile start would wipe earlier tiles whose pipelined
                    # Q-matmuls land later. Pending-zero propagation makes the
                    # first write to each slice an overwrite either way.
                    nc.tensor.matmul(psum_h1[:, tt, :], xself[:, 2 * tt:2 * tt + 2, :],
                                     w1sb[:, :, :], start=(tt == 0), stop=(ft == 0),
                                     perf_mode=DR)
                    for j in range(ft):
                        cc = c % 4
                        if cc == 0:
                            psum_A = psA.tile([128, 4, HID], f32, name="psum_A", tag="A")
                            xws_grp = wpool.tile([128, 4, HID], fp8, name="xws_grp")
                        if c % 8 == 0:
                            nld = min(8, C - c)
                            xt2 = spool.tile([128, 24, 128], fp8, name="xt2")
                            next_eng().dma_start(xt2[:, 0:nld * 3, :],
                                                 xs[:, c * 3:(c + nld) * 3, :])
                        sl = (c % 8) * 3
                        nc.tensor.matmul(psum_A[:, cc, :], xt2[:, sl:sl + 2, :],
                                         w1sb[:, :, :], start=True, stop=True,
                                         perf_mode=DR)
                        pend.append((xt2[:, sl + 2, :], xws_grp, cc, psum_h1, tt,
                                     j == ft - 1))
                        if cc == 3 or c == C - 1:
                            flush_pend(cc + 1)
                        c += 1
                    if tt == 3 or t == T_TILES - 1:
                        flush_pend(((c - 1) % 4) + 1 if pend else 0, drain=True)
                        ns = tt + 1
                        tmp = wpool.tile([128, 4, HID], f32, name="h1tmp")
                        nc.vector.tensor_tensor(tmp[:, 0:ns, :], psum_h1[:, 0:ns, :],
                                                b1b4[:, 0:ns, :], op=mybir.AluOpType.add)
                        nc.scalar.activation(h1_grp[:, 0:ns, :], tmp[:, 0:ns, :],
                                             mybir.ActivationFunctionType.Relu)
                        t0 = t - tt
                        # G += h1-pair (x) M-pair, one DoubleRow per 2 tiles,
                        # one 4-tile group delayed so PE never waits on relu
                        for (h1p, pgi, pns) in pend_m:
                            for jj in range(0, pns, 2):
                                nc.tensor.matmul(
                                    psum_G[:], h1p[:, jj:jj + 2, :],
                                    mt_t[pgi][:, jj:jj + 2, :],
                                    start=(pgi == 0 and jj == 0), stop=False,
                                    perf_mode=DR)
                        pend_m = [(h1_grp, gi, ns)]
                        if t == T_TILES - 1:
                            for (h1p, pgi, pns) in pend_m:
                                for jj in range(0, pns, 2):
                                    nc.tensor.matmul(
                                        psum_G[:], h1p[:, jj:jj + 2, :],
                                        mt_t[pgi][:, jj:jj + 2, :],
                                        start=(pgi == 0 and jj == 0),
                                        stop=(pgi * 4 + jj + 2 >= T_TILES),
                                        perf_mode=DR)
                            pend_m = []
                col = 0 if bn == "td" else B
                nc.vector.tensor_copy(gsb[:, col:col + B], psum_G[:])
                if bn == "td":
                    # head weights: prefetch mid-program, off the ramp path
                    pw1 = cpool.tile([128, 2, 2 * HID], f32)
                    nc.gpsimd.dma_start(pw1[:], dram_in["pw1"].rearrange("(kc p) n -> p kc n", p=128))
                    pb1 = cpool.tile([128, 2], f32)
                    nc.gpsimd.dma_start(pb1[:], dram_in["pb1"][:, :])
                    pw2 = cpool.tile([128, 2, HID], f32)
                    nc.gpsimd.dma_start(pw2[:], dram_in["pw2"].rearrange("(kc p) n -> p kc n", p=128))
                    pb2 = cpool.tile([128, 1], f32)
                    nc.gpsimd.dma_start(pb2[:], dram_in["pb2"][:, :])
                    w2sb = {}
                    pbsb = {}
                    for bn2 in ("td", "bu"):
                        w2sb[bn2] = cpool.tile([HID, HID], f32, name=f"w2sb_{bn2}")
                        nc.sync.dma_start(w2sb[bn2][:], dram_in[f"w2_{bn2}"][:, :])
                        pbsb[bn2] = cpool.tile([HID, B], f32, name=f"pbsb_{bn2}")
                        nc.scalar.dma_start(pbsb[bn2][:], dram_in[f"pb_{bn2}"][:, :])

            # ---- single combined AllReduce (td|bu concatenated, 128KB) ----
            arin = dpool.tile([HID, 2 * B], f32, name="arin")
            arout = dpool.tile([HID, 2 * B], f32, addr_space="Shared", name="arout")
            nc.gpsimd.dma_start(arin[:], gsb[:])
            nc.gpsimd.collective_compute(
                "AllReduce", mybir.AluOpType.add,
                replica_groups=[list(range(N_CORES))],
                ins=[arin[:]], outs=[arout[:]],
            )
            garr = cpool.tile([HID, 2 * B], f32, name="garr")
            nc.gpsimd.dma_start(garr[:], arout[:])

            # ---- MLP head (replicated on every core, transposed layout) ----
            pool_t = {}
            for i, bn in enumerate(("td", "bu")):
                ps_p = psA.tile([HID, B], f32, name="ps_p", tag="A")
                nc.tensor.matmul(ps_p[:], w2sb[bn][:], garr[:, i * B:(i + 1) * B],
                                 start=True, stop=True)
                pt = cpool.tile([HID, B], f32, name=f"pool_{bn}")
                nc.vector.tensor_tensor(pt[:], ps_p[:], pbsb[bn][:], op=mybir.AluOpType.add)
                pool_t[bn] = pt                                      # pooled^T [f, g]

            r1 = []
            for hh in range(2):
                ps1 = psA.tile([128, B], f32, name="ps1", tag="A")
                nc.tensor.matmul(ps1[:], pw1[:, 0, hh * 128:(hh + 1) * 128],
                                 pool_t["bu"][:], start=True, stop=False)
                nc.tensor.matmul(ps1[:], pw1[:, 1, hh * 128:(hh + 1) * 128],
                                 pool_t["td"][:], start=False, stop=True)
                r = wpool.tile([128, B], f32, name=f"r1_{hh}")
                nc.scalar.activation(r[:], ps1[:], mybir.ActivationFunctionType.Relu,
                                     bias=pb1[:, hh:hh + 1])
                r1.append(r)
            ps2 = psH.tile([HID, B], f32, name="ps2", tag="H")
            for hh in range(2):
                nc.tensor.matmul(ps2[:], pw2[:, hh, :], r1[hh][:],
                                 start=(hh == 0), stop=(hh == 1))
            ofin = wpool.tile([HID, B], f32, name="ofin")
            nc.vector.tensor_scalar(ofin[:], ps2[:], pb2[:, 0:1], None,
                                    op0=mybir.AluOpType.add)
            nc.gpsimd.dma_start(out_t[:, :], ofin[:])

    _split_excess_waits(nc, limit=1)
    return nc


# ------------------------------------------------------------------- staging
def _stage_core(k, xvp_f32, br, counts_g, inputs, np_fp8):
    m = {}
    for bn in ("td", "bu"):
        d = br[bn]
        C = d["C"]
        tiles = d["tile_at"][k]                         # [98] global tile ids

        # edge stream blob [128, C*3, 128]: per chunk 3 slices (x-k0, x-k1, Q)
        src = d["ent_src"][k]
        nrm = d["ent_norm"][k]
        xg = xvp_f32[src] * nrm[:, None]                # [C*128, 256] f32
        xpart = xg.reshape(C, 128, IN).transpose(2, 0, 1)   # [256, C, 128]
        xpart = xpart.reshape(2, 128, C, 128).transpose(1, 2, 0, 3)  # [128,C,2,128]
        slot = d["ent_slot"][k]
        Q = np.zeros((C, 128, 128), dtype=np.float32)
        Q.reshape(C * 128, 128)[np.arange(C * 128), slot] = 1.0
        qpart = Q.transpose(1, 0, 2)                    # [128, C, 128]
        blob = np.concatenate([xpart, qpart[:, :, None, :]], axis=2)  # [128,C,3,128]
        m[f"xs_{bn}"] = np.ascontiguousarray(
            blob.reshape(128, C * 3, 128), dtype=np_fp8)

        # dense self block [128, 98*2, 128]: xselfT[feat, tile, kc, node]
        nodes = (tiles[:, None] * 128 + np.arange(128)[None, :]).reshape(-1)
        xs_blk = xvp_f32[nodes] * d["dinv2"][np.minimum(nodes, NV - 1)][:, None]
        xs_blk[nodes >= NV] = 0.0
        A = xs_blk.reshape(T_TILES, 128, IN).transpose(2, 0, 1)  # [256, 98, 128]
        A = A.reshape(2, 128, T_TILES, 128).transpose(1, 2, 0, 3)  # [128,98,2,128]
        m[f"xself_{bn}"] = np.ascontiguousarray(
            A.reshape(128, T_TILES * 2, 128), dtype=np_fp8)

        # M^T columns for this core's tiles, [128, 98, 128] = [node, tile, graph]
        Mc = d["M"][:, nodes].reshape(B, T_TILES, 128)  # [g, t, p]
        m[f"mt_{bn}"] = np.ascontiguousarray(Mc.transpose(2, 1, 0), dtype=np_fp8)

        w1 = np.asarray(inputs[f"{bn}_w1"], np.float32) * W1_SCALE
        m[f"w1_{bn}"] = np.ascontiguousarray(
            w1.reshape(2, 128, HID).transpose(1, 0, 2), dtype=np_fp8)
        b1b = np.broadcast_to(
            np.asarray(inputs[f"{bn}_b1"], np.float32) * W1_SCALE, (128, 4, HID))
        m[f"b1b_{bn}"] = np.ascontiguousarray(b1b, dtype=np.float32)
        m[f"w2_{bn}"] = np.ascontiguousarray(
            np.asarray(inputs[f"{bn}_w2"], np.float32) / W1_SCALE)
        m[f"pb_{bn}"] = np.ascontiguousarray(
            np.outer(np.asarray(inputs[f"{bn}_b2"], np.float64), counts_g + 1.0),
            dtype=np.float32)
    m["pw1"] = np.ascontiguousarray(np.asarray(inputs["p_w1"], np.float32))
    m["pb1"] = np.ascontiguousarray(
        np.asarray(inputs["p_b1"], np.float32).reshape(2, 128).T)
    m["pw2"] = np.ascontiguousarray(np.asarray(inputs["p_w2"], np.float32))
    m["pb2"] = np.asarray(inputs["p_b2"], np.float32).reshape(128, 1).copy()
    return m


def _run(inputs, trace=False):
    import ml_dtypes
    from concourse import bass_utils

    x = np.asarray(inputs["x"])
    edge_index = np.asarray(inputs["edge_index"])
    batch = np.asarray(inputs["batch"])
    xv, br, counts_g = _host_prep(x, inputs["emb_w"], edge_index, batch)
    xvp = np.zeros((NVP, IN), dtype=np.float32)
    xvp[:NV] = xv

    np_fp8 = ml_dtypes.float8_e4m3
    in_maps = [_stage_core(k, xvp, br, counts_g, inputs, np_fp8)
               for k in range(N_CORES)]
    nc = _build_program(br["td"]["F"], br["bu"]["F"])
    last = None
    for attempt in range(3):
        try:
            res = bass_utils.run_bass_kernel_spmd(
                nc, in_maps, core_ids=list(range(N_CORES)), trace=trace)
            break
        except Exception as e:   # transient NRT device errors recover on retry
            last = e
    else:
        raise last
    out = np.ascontiguousarray(res.results[0]["outT"].T, dtype=np.float32)
    return out, res


def kernel(**inputs) -> np.ndarray:
    out, _ = _run(inputs, trace=False)
    return out


# revision 16
# speedup vs baseline: 1.1657x; 1.1567x over previous
"""BiGCN (graphcl) Trainium2 kernel — 8-core SPMD, fp8 DoubleRow edition.

Decomposition (per branch, A = sym-normalized adjacency with self loops):
    h1     = relu(A @ (xv @ W1) + b1)
    pooled = M @ h1 @ W2 + (c_g + 1) * b2        with M = T @ A (dense [B, nv])
    h      = [bu | td];  out = relu(h @ p_w1 + p_b1) @ p_w2 + p_b2

Sharding: 128-node tiles are assigned to (core, position) by a balanced
snake deal over per-tile edge-chunk counts, so the shared SPMD program's
per-position chunk count F[pos] (maxed over cores) wastes almost nothing.

Per tile, layer 1 splits into
  self-loop part: one fp8 DoubleRow matmul from a dense preloaded block
      xselfT[feat, node] * dinv2  ->  psum_h1 (start)
  edge part: host-staged per-core edge streams (gathered source rows
      pre-multiplied by norm, fp8, partition-major so every DMA reads
      >=2KB contiguous per partition). Per chunk:
      x_chunk fp8 DoubleRow (K=256 in one op) -> psum_A -> cast fp8
      psum_h1 += Q^T @ xw  (one-hot Q streamed in the same blob)
Layer 2 + pooling collapse into G += [h1 pair]^T (x) [M pair] fp8
DoubleRow pairs accumulated in one PSUM bank; G is the only collective
(64KB AllReduce), then the tiny MLP head runs replicated on every core.

fp8 scale trick: W1, b1 are staged x16 (relu is positively homogeneous),
W2 staged /16 — keeps W1 out of fp8's subnormal range.
"""
import numpy as np

N_CORES = 8
N = 100000
NV = N + 1
S = 12544                 # nodes per core = 98 * 128
T_TILES = S // 128        # 98
G_TILES = N_CORES * T_TILES   # 784 global tiles
NVP = N_CORES * S
B = 128
IN = 256
HID = 128

W1_SCALE = 16.0
DEBUG = False


# ----------------------------------------------------------------- host prep
def _build_branch(s_e, d_e, batch):
    """s_e/d_e: edge endpoints WITHOUT self loops (E real + B virtual edges).

    Returns per-branch staging data. Self loops enter deg, M and the dense
    per-tile self blocks, not the edge stream.
    """
    deg = np.bincount(d_e, minlength=NV).astype(np.float64) + 1.0
    dinv = 1.0 / np.sqrt(deg)
    enorm = (dinv[s_e] * dinv[d_e]).astype(np.float64)

    # M = T @ A over full A (edges + self loops)
    es = np.concatenate([s_e, np.arange(NV, dtype=np.int64)])
    ed = np.concatenate([d_e, np.arange(NV, dtype=np.int64)])
    en = np.concatenate([enorm, 1.0 / deg])
    M = np.zeros((B, NVP), dtype=np.float64)
    real = ed < N
    np.add.at(M, (batch[ed[real]].astype(np.int64), es[real]), en[real])
    virt = ~real
    if virt.any():
        M += np.bincount(es[virt], weights=en[virt], minlength=NVP)[None, :]

    # balanced tile -> (core, pos) assignment over edge-chunk counts
    gt = d_e // 128
    counts = np.bincount(gt, minlength=G_TILES)
    ct = -(-counts // 128)                      # ceil, 0 for empty tiles
    order = np.argsort(-ct, kind="stable")      # tiles by count desc
    tile_at = np.empty((N_CORES, T_TILES), dtype=np.int64)
    for r, tl in enumerate(order):
        row, idx = divmod(r, N_CORES)
        core = idx if (row % 2 == 0) else N_CORES - 1 - idx
        tile_at[core, row] = tl
    F = ct[order[::N_CORES]].astype(np.int64)   # per-position max over cores
    C = int(F.sum())
    chunk_base = np.concatenate([[0], np.cumsum(F)])

    # group edge entries by global tile
    eorder = np.argsort(gt, kind="stable")
    es_s, ed_s, en_s = s_e[eorder], d_e[eorder], enorm[eorder]
    tile_starts = np.concatenate([[0], np.cumsum(counts)])

    ent_src = np.zeros((N_CORES, C * 128), dtype=np.int64)
    ent_norm = np.zeros((N_CORES, C * 128), dtype=np.float32)
    ent_slot = np.zeros((N_CORES, C * 128), dtype=np.int64)
    for k in range(N_CORES):
        for t in range(T_TILES):
            tl = tile_at[k, t]
            a, bnd = tile_starts[tl], tile_starts[tl + 1]
            if bnd == a:
                continue
            off = chunk_base[t] * 128
            m = bnd - a
            ent_src[k, off:off + m] = es_s[a:bnd]
            ent_norm[k, off:off + m] = en_s[a:bnd]
            ent_slot[k, off:off + m] = ed_s[a:bnd] - tl * 128
    return dict(ent_src=ent_src, ent_norm=ent_norm, ent_slot=ent_slot,
                F=F, C=C, M=M, tile_at=tile_at, dinv2=(1.0 / deg))


def _host_prep(x, emb_w, edge_index, batch):
    xv = np.concatenate([np.asarray(x, np.float32),
                         np.asarray(emb_w, np.float32)], axis=0)
    roots = np.searchsorted(batch, np.arange(B, dtype=batch.dtype)).astype(np.int64)
    ei0 = edge_index[0].astype(np.int64)
    ei1 = edge_index[1].astype(np.int64)
    vs = np.full(B, N, dtype=np.int64)
    br = {
        "td": _build_branch(np.concatenate([ei0, vs]), np.concatenate([ei1, roots]), batch),
        "bu": _build_branch(np.concatenate([ei1, roots]), np.concatenate([ei0, vs]), batch),
    }
    counts_g = np.bincount(batch, minlength=B).astype(np.float64)
    return xv, br, counts_g


# ------------------------------------------------------- walrus wait limiter
def _split_excess_waits(nc, limit=1):
    import concourse.mybir as mybir
    n_added = 0
    for bb in nc.main_func.blocks:
        insts = bb.instructions
        new_list = []
        for inst in insts:
            si = inst.sync_info
            if si is not None and si.on_wait and len(si.on_wait) > limit:
                waits = list(si.on_wait)
                extra, keep = waits[:-limit], waits[-limit:]
                for w in extra:
                    noop = mybir.InstNoOp(name=f"I-wsplit-{nc.next_id()}", ins=[], outs=[])
                    noop.engine = inst.engine
                    noop.sync_info = mybir.SyncInfo(on_wait=[w], on_update=[])
                    nc.register_instruction(noop, overwrite=True)
                    new_list.append(noop)
                    n_added += 1
                inst.sync_info = mybir.SyncInfo(on_wait=keep, on_update=list(si.on_update or []))
            new_list.append(inst)
        insts[:] = new_list
    return n_added


# ------------------------------------------------------------ device program
def _build_program(F_td, F_bu):
    import concourse.bass as bass
    import concourse.mybir as mybir
    import concourse.tile as tile

    f32 = mybir.dt.float32
    fp8 = mybir.dt.float8e4
    DR = mybir.MatmulPerfMode.DoubleRow

    nc = bass.Bass(target_bir_lowering=False, trn_type="TRN2", num_swdge_queues=4)

    dram_in = {}
    for bn, C in (("td", int(F_td.sum())), ("bu", int(F_bu.sum()))):
        dram_in[f"xs_{bn}"] = nc.dram_tensor(f"xs_{bn}", [128, C * 3, 128], fp8, kind="ExternalInput")
        dram_in[f"xself_{bn}"] = nc.dram_tensor(f"xself_{bn}", [128, T_TILES * 2, 128], fp8, kind="ExternalInput")
        dram_in[f"mt_{bn}"] = nc.dram_tensor(f"mt_{bn}", [128, T_TILES, 128], fp8, kind="ExternalInput")
        dram_in[f"w1_{bn}"] = nc.dram_tensor(f"w1_{bn}", [128, 2, HID], fp8, kind="ExternalInput")
        dram_in[f"b1b_{bn}"] = nc.dram_tensor(f"b1b_{bn}", [128, 4, HID], f32, kind="ExternalInput")
        dram_in[f"w2_{bn}"] = nc.dram_tensor(f"w2_{bn}", [HID, HID], f32, kind="ExternalInput")
        dram_in[f"pb_{bn}"] = nc.dram_tensor(f"pb_{bn}", [HID, B], f32, kind="ExternalInput")
    dram_in["pw1"] = nc.dram_tensor("pw1", [2 * HID, 2 * HID], f32, kind="ExternalInput")
    dram_in["pb1"] = nc.dram_tensor("pb1", [128, 2], f32, kind="ExternalInput")
    dram_in["pw2"] = nc.dram_tensor("pw2", [2 * HID, HID], f32, kind="ExternalInput")
    dram_in["pb2"] = nc.dram_tensor("pb2", [128, 1], f32, kind="ExternalInput")
    out_t = nc.dram_tensor("outT", [HID, B], f32, kind="ExternalOutput")

    N_GRP = (T_TILES + 3) // 4

    with tile.TileContext(nc) as tc:
        with (
            tc.tile_pool(name="const", bufs=1) as cpool,
            tc.tile_pool(name="stream", bufs=6) as spool,
            tc.tile_pool(name="selfp", bufs=3) as selfpool,
            tc.tile_pool(name="mtp", bufs=3) as mtpool,
            tc.tile_pool(name="work", bufs=8) as wpool,
            tc.tile_pool(name="psA", bufs=3, space="PSUM") as psA,
            tc.tile_pool(name="psH", bufs=3, space="PSUM") as psH,
            tc.tile_pool(name="psG", bufs=2, space="PSUM") as psG,
            tc.tile_pool(name="dram", bufs=1, space="DRAM") as dpool,
        ):
            dma_engines = [nc.sync, nc.scalar, nc.gpsimd]
            dma_rr = [0]

            def next_eng():
                eng = dma_engines[dma_rr[0] % len(dma_engines)]
                dma_rr[0] += 1
                return eng

            # ---- per-branch small constants -----------------------------
            consts = {}
            for bn in ("td", "bu"):
                w1sb = cpool.tile([128, 2, HID], fp8, name=f"w1sb_{bn}")
                nc.sync.dma_start(w1sb[:], dram_in[f"w1_{bn}"][:, :, :])
                b1b4 = cpool.tile([128, 4, HID], f32, name=f"b1b4_{bn}")
                nc.scalar.dma_start(b1b4[:], dram_in[f"b1b_{bn}"][:, :, :])
                consts[bn] = (w1sb, b1b4)

            gsb = cpool.tile([HID, 2 * B], f32, name="gsb")   # [f, td|bu graphs]
            for bn, F in (("td", F_td), ("bu", F_bu)):
                C = int(F.sum())
                xs = dram_in[f"xs_{bn}"]
                xself_d = dram_in[f"xself_{bn}"]
                mt_d = dram_in[f"mt_{bn}"]
                w1sb, b1b4 = consts[bn]

                # just-in-time per-group xself/mt slices (1 group lookahead)
                xself_t = {}
                mt_t = {}

                def load_group(g):
                    if g >= N_GRP or g in xself_t:
                        return
                    ns_g = min(4, T_TILES - g * 4)
                    xt = selfpool.tile([128, 8, 128], fp8, name="xselfg")
                    next_eng().dma_start(xt[:, 0:2 * ns_g, :],
                                         xself_d[:, g * 8:g * 8 + 2 * ns_g, :])
                    mtt = mtpool.tile([128, 4, 128], fp8, name="mtg")
                    next_eng().dma_start(mtt[:, 0:ns_g, :],
                                         mt_d[:, g * 4:g * 4 + ns_g, :])
                    xself_t[g] = xt
                    mt_t[g] = mtt

                load_group(0)
                load_group(1)

                psum_G = psG.tile([HID, B], f32, name=f"psum_G_{bn}", tag="G")

                xws_grp = None
                psum_A = None
                cast_rr = [0]

                def emit_qmms(lst):
                    # merge eligible (same tile, adjacent chunk) pairs into
                    # one fp8 DoubleRow Q-matmul (strided lhsT view)
                    k = 0
                    while k < len(lst):
                        (xt, sl, xg, cc2, ph, tt2, sp) = lst[k]
                        if k + 1 < len(lst):
                            (xt_n, sl_n, xg_n, cc_n, ph_n, tt_n, sp_n) = lst[k + 1]
                            if (xt_n is xt and sl_n == sl + 3 and xg_n is xg
                                    and cc_n == cc2 + 1 and ph_n is ph
                                    and tt_n == tt2):
                                nc.tensor.matmul(ph[:, tt2, :],
                                                 xt[:, sl + 2:sl + 6:3, :],
                                                 xg[:, cc2:cc2 + 2, :],
                                                 start=False, stop=sp_n,
                                                 perf_mode=DR)
                                k += 2
                                continue
                        nc.tensor.matmul(ph[:, tt2, :], xt[:, sl + 2, :],
                                         xg[:, cc2, :], start=False, stop=sp)
                        k += 1

                def flush_pend(nslices, drain=False):
                    # cast current group (alternating DVE/ACT so neither
                    # saturates), emit the group-before-previous Q-matmuls
                    # (2-group software pipeline so PE never waits on casts)
                    if pend:
                        if cast_rr[0] % 2 == 0:
                            nc.vector.tensor_copy(xws_grp[:, 0:nslices, :],
                                                  psum_A[:, 0:nslices, :])
                        else:
                            nc.scalar.activation(xws_grp[:, 0:nslices, :],
                                                 psum_A[:, 0:nslices, :],
                                                 mybir.ActivationFunctionType.Copy)
                        cast_rr[0] += 1
                        emit_qmms(pend_prev2)
                        pend_prev2[:] = list(pend_prev)
                        pend_prev[:] = list(pend)
                        pend.clear()
                    if drain:
                        emit_qmms(pend_prev2 + pend_prev)
                        pend_prev2.clear()
                        pend_prev.clear()

                c = 0
                h1_grp = None
                psum_h1 = None
                xt2 = None
                pend: list = []
                pend_prev: list = []
                pend_prev2: list = []
                pend_m: list = []
                for t in range(T_TILES):
                    tt = t % 4
                    gi = t // 4
                    if tt == 0:
                        psum_h1 = psH.tile([128, 4, HID], f32, name="psum_h1", tag="H")
                        h1_grp = wpool.tile([128, 4, HID], fp8, name="h1_grp")
                        load_group(gi + 1)
                    ft = int(F[t])
                    xself = xself_t[gi]
                    # self-loop part: one DoubleRow, first write of this tile's
                    # psum slice. start=True only on the group's first matmul:
                    # start marks the WHOLE 2KB psum bank pending-zero, so a
                    # per-tile start would wipe earlier tiles whose pipelined
                    # Q-matmuls land later. Pending-zero propagation makes the
                    # first write to each slice an overwrite either way.
                    nc.tensor.matmul(psum_h1[:, tt, :], xself[:, 2 * tt:2 * tt + 2, :],
                                     w1sb[:, :, :], start=(tt == 0), stop=(ft == 0),
                                     perf_mode=DR)
                    for j in range(ft):
                        cc = c % 4
                        if cc == 0:
                            psum_A = psA.tile([128, 4, HID], f32, name="psum_A", tag="A")
                            xws_grp = wpool.tile([128, 4, HID], fp8, name="xws_grp")
                        if c % 8 == 0:
                            nld = min(8, C - c)
                            xt2 = spool.tile([128, 24, 128], fp8, name="xt2")
                            next_eng().dma_start(xt2[:, 0:nld * 3, :],
                                                 xs[:, c * 3:(c + nld) * 3, :])
                        sl = (c % 8) * 3
                        nc.tensor.matmul(psum_A[:, cc, :], xt2[:, sl:sl + 2, :],
                                         w1sb[:, :, :], start=True, stop=True,
                                         perf_mode=DR)
                        pend.append((xt2, sl, xws_grp, cc, psum_h1, tt,
                                     j == ft - 1))
                        if cc == 3 or c == C - 1:
                            flush_pend(cc + 1)
                        c += 1
                    if tt == 3 or t == T_TILES - 1:
                        flush_pend(((c - 1) % 4) + 1 if pend else 0, drain=True)
                        ns = tt + 1
                        tmp = wpool.tile([128, 4, HID], f32, name="h1tmp")
                        nc.vector.tensor_tensor(tmp[:, 0:ns, :], psum_h1[:, 0:ns, :],
                                                b1b4[:, 0:ns, :], op=mybir.AluOpType.add)
                        nc.scalar.activation(h1_grp[:, 0:ns, :], tmp[:, 0:ns, :],
                                             mybir.ActivationFunctionType.Relu)
                        t0 = t - tt
                        # G += h1-pair (x) M-pair, one DoubleRow per 2 tiles,
                        # one 4-tile group delayed so PE never waits on relu
                        for (h1p, pgi, pns) in pend_m:
                            for jj in range(0, pns, 2):
                                nc.tensor.matmul(
                                    psum_G[:], h1p[:, jj:jj + 2, :],
                                    mt_t[pgi][:, jj:jj + 2, :],
                                    start=(pgi == 0 and jj == 0), stop=False,
                                    perf_mode=DR)
                        pend_m = [(h1_grp, gi, ns)]
                        if t == T_TILES - 1:
                            for (h1p, pgi, pns) in pend_m:
                                for jj in range(0, pns, 2):
                                    nc.tensor.matmul(
                                        psum_G[:], h1p[:, jj:jj + 2, :],
                                        mt_t[pgi][:, jj:jj + 2, :],
                                        start=(pgi == 0 and jj == 0),
                                        stop=(pgi * 4 + jj + 2 >= T_TILES),
                                        perf_mode=DR)
                            pend_m = []
                col = 0 if bn == "td" else B
                nc.vector.tensor_copy(gsb[:, col:col + B], psum_G[:])
                if bn == "td":
                    # head weights: prefetch mid-program, off the ramp path
                    pw1 = cpool.tile([128, 2, 2 * HID], f32)
                    nc.gpsimd.dma_start(pw1[:], dram_in["pw1"].rearrange("(kc p) n -> p kc n", p=128))
                    pb1 = cpool.tile([128, 2], f32)
                    nc.gpsimd.dma_start(pb1[:], dram_in["pb1"][:, :])
                    pw2 = cpool.tile([128, 2, HID], f32)
                    nc.gpsimd.dma_start(pw2[:], dram_in["pw2"].rearrange("(kc p) n -> p kc n", p=128))
                    pb2 = cpool.tile([128, 1], f32)
                    nc.gpsimd.dma_start(pb2[:], dram_in["pb2"][:, :])
                    w2sb = {}
                    pbsb = {}
                    for bn2 in ("td", "bu"):
                        w2sb[bn2] = cpool.tile([HID, HID], f32, name=f"w2sb_{bn2}")
                        nc.sync.dma_start(w2sb[bn2][:], dram_in[f"w2_{bn2}"][:, :])
                        pbsb[bn2] = cpool.tile([HID, B], f32, name=f"pbsb_{bn2}")
                        nc.scalar.dma_start(pbsb[bn2][:], dram_in[f"pb_{bn2}"][:, :])

            # ---- single combined AllReduce (td|bu concatenated, 128KB) ----
            arin = dpool.tile([HID, 2 * B], f32, name="arin")
            arout = dpool.tile([HID, 2 * B], f32, addr_space="Shared", name="arout")
            nc.gpsimd.dma_start(arin[:], gsb[:])
            nc.gpsimd.collective_compute(
                "AllReduce", mybir.AluOpType.add,
                replica_groups=[list(range(N_CORES))],
                ins=[arin[:]], outs=[arout[:]],
            )
            garr = cpool.tile([HID, 2 * B], f32, name="garr")
            nc.gpsimd.dma_start(garr[:], arout[:])

            # ---- MLP head (replicated on every core, transposed layout) ----
            pool_t = {}
            for i, bn in enumerate(("td", "bu")):
                ps_p = psA.tile([HID, B], f32, name="ps_p", tag="A")
                nc.tensor.matmul(ps_p[:], w2sb[bn][:], garr[:, i * B:(i + 1) * B],
                                 start=True, stop=True)
                pt = cpool.tile([HID, B], f32, name=f"pool_{bn}")
                nc.vector.tensor_tensor(pt[:], ps_p[:], pbsb[bn][:], op=mybir.AluOpType.add)
                pool_t[bn] = pt                                      # pooled^T [f, g]

            r1 = []
            for hh in range(2):
                ps1 = psA.tile([128, B], f32, name="ps1", tag="A")
                nc.tensor.matmul(ps1[:], pw1[:, 0, hh * 128:(hh + 1) * 128],
                                 pool_t["bu"][:], start=True, stop=False)
                nc.tensor.matmul(ps1[:], pw1[:, 1, hh * 128:(hh + 1) * 128],
                                 pool_t["td"][:], start=False, stop=True)
                r = wpool.tile([128, B], f32, name=f"r1_{hh}")
                nc.scalar.activation(r[:], ps1[:], mybir.ActivationFunctionType.Relu,
                                     bias=pb1[:, hh:hh + 1])
                r1.append(r)
            ps2 = psH.tile([HID, B], f32, name="ps2", tag="H")
            for hh in range(2):
                nc.tensor.matmul(ps2[:], pw2[:, hh, :], r1[hh][:],
                                 start=(hh == 0), stop=(hh == 1))
            ofin = wpool.tile([HID, B], f32, name="ofin")
            nc.vector.tensor_scalar(ofin[:], ps2[:], pb2[:, 0:1], None,
                                    op0=mybir.AluOpType.add)
            nc.gpsimd.dma_start(out_t[:, :], ofin[:])

    _split_excess_waits(nc, limit=1)
    return nc


# ------------------------------------------------------------------- staging
def _stage_core(k, xvp_f32, br, counts_g, inputs, np_fp8):
    m = {}
    for bn in ("td", "bu"):
        d = br[bn]
        C = d["C"]
        tiles = d["tile_at"][k]                         # [98] global tile ids

        # edge stream blob [128, C*3, 128]: per chunk 3 slices (x-k0, x-k1, Q)
        src = d["ent_src"][k]
        nrm = d["ent_norm"][k]
        xg = xvp_f32[src] * nrm[:, None]                # [C*128, 256] f32
        xpart = xg.reshape(C, 128, IN).transpose(2, 0, 1)   # [256, C, 128]
        xpart = xpart.reshape(2, 128, C, 128).transpose(1, 2, 0, 3)  # [128,C,2,128]
        slot = d["ent_slot"][k]
        Q = np.zeros((C, 128, 128), dtype=np.float32)
        Q.reshape(C * 128, 128)[np.arange(C * 128), slot] = 1.0
        qpart = Q.transpose(1, 0, 2)                    # [128, C, 128]
        blob = np.concatenate([xpart, qpart[:, :, None, :]], axis=2)  # [128,C,3,128]
        m[f"xs_{bn}"] = np.ascontiguousarray(
            blob.reshape(128, C * 3, 128), dtype=np_fp8)

        # dense self block [128, 98*2, 128]: xselfT[feat, tile, kc, node]
        nodes = (tiles[:, None] * 128 + np.arange(128)[None, :]).reshape(-1)
        xs_blk = xvp_f32[nodes] * d["dinv2"][np.minimum(nodes, NV - 1)][:, None]
        xs_blk[nodes >= NV] = 0.0
        A = xs_blk.reshape(T_TILES, 128, IN).transpose(2, 0, 1)  # [256, 98, 128]
        A = A.reshape(2, 128, T_TILES, 128).transpose(1, 2, 0, 3)  # [128,98,2,128]
        m[f"xself_{bn}"] = np.ascontiguousarray(
            A.reshape(128, T_TILES * 2, 128), dtype=np_fp8)

        # M^T columns for this core's tiles, [128, 98, 128] = [node, tile, graph]
        Mc = d["M"][:, nodes].reshape(B, T_TILES, 128)  # [g, t, p]
        m[f"mt_{bn}"] = np.ascontiguousarray(Mc.transpose(2, 1, 0), dtype=np_fp8)

        w1 = np.asarray(inputs[f"{bn}_w1"], np.float32) * W1_SCALE
        m[f"w1_{bn}"] = np.ascontiguousarray(
            w1.reshape(2, 128, HID).transpose(1, 0, 2), dtype=np_fp8)
        b1b = np.broadcast_to(
            np.asarray(inputs[f"{bn}_b1"], np.float32) * W1_SCALE, (128, 4, HID))
        m[f"b1b_{bn}"] = np.ascontiguousarray(b1b, dtype=np.float32)
        m[f"w2_{bn}"] = np.ascontiguousarray(
            np.asarray(inputs[f"{bn}_w2"], np.float32) / W1_SCALE)
        m[f"pb_{bn}"] = np.ascontiguousarray(
            np.outer(np.asarray(inputs[f"{bn}_b2"], np.float64), counts_g + 1.0),
            dtype=np.float32)
    m["pw1"] = np.ascontiguousarray(np.asarray(inputs["p_w1"], np.float32))
    m["pb1"] = np.ascontiguousarray(
        np.asarray(inputs["p_b1"], np.float32).reshape(2, 128).T)
    m["pw2"] = np.ascontiguousarray(np.asarray(inputs["p_w2"], np.float32))
    m["pb2"] = np.asarray(inputs["p_b2"], np.float32).reshape(128, 1).copy()
    return m


def _enable_ldw_opt():
    import os, stat, tempfile
    from concourse import bass_utils
    if getattr(bass_utils, "_ldw_shim", None):
        return
    real = bass_utils.get_walrus_driver()
    shim = os.path.join(tempfile.gettempdir(), "walrus_ldw_shim.sh")
    with open(shim, "w") as f:
        f.write("#!/bin/sh\nargs=\"\"\nfor a in \"$@\"; do\n"
                "  case \"$a\" in --enable-ldw-opt=false) a=--enable-ldw-opt=true;; esac\n"
                "  args=\"$args $a\"\ndone\nexec %s $args\n" % real)
    os.chmod(shim, stat.S_IRWXU)
    bass_utils.get_walrus_driver = lambda: shim
    bass_utils._ldw_shim = shim


def _run(inputs, trace=False):
    import ml_dtypes
    from concourse import bass_utils
    # NOTE: walrus --enable-ldw-opt=true rejects DoubleRow ldweights
    # ("InstLdweights is not compatible with LDW optimization") — keep off.

    x = np.asarray(inputs["x"])
    edge_index = np.asarray(inputs["edge_index"])
    batch = np.asarray(inputs["batch"])
    xv, br, counts_g = _host_prep(x, inputs["emb_w"], edge_index, batch)
    xvp = np.zeros((NVP, IN), dtype=np.float32)
    xvp[:NV] = xv

    np_fp8 = ml_dtypes.float8_e4m3
    in_maps = [_stage_core(k, xvp, br, counts_g, inputs, np_fp8)
               for k in range(N_CORES)]
    nc = _build_program(br["td"]["F"], br["bu"]["F"])
    last = None
    for attempt in range(3):
        try:
            res = bass_utils.run_bass_kernel_spmd(
                nc, in_maps, core_ids=list(range(N_CORES)), trace=trace)
            break
        except Exception as e:   # transient NRT device errors recover on retry
            last = e
    else:
        raise last
    out = np.ascontiguousarray(res.results[0]["outT"].T, dtype=np.float32)
    return out, res


def kernel(**inputs) -> np.ndarray:
    out, _ = _run(inputs, trace=False)
    return out


# revision 22
# speedup vs baseline: 1.2187x; 1.0455x over previous
"""BiGCN (graphcl) Trainium2 kernel — 8-core SPMD, fp8 DoubleRow edition.

Decomposition (per branch, A = sym-normalized adjacency with self loops):
    h1     = relu(A @ (xv @ W1) + b1)
    pooled = M @ h1 @ W2 + (c_g + 1) * b2        with M = T @ A (dense [B, nv])
    h      = [bu | td];  out = relu(h @ p_w1 + p_b1) @ p_w2 + p_b2

Sharding: 128-node tiles are assigned to (core, position) by a balanced
snake deal over per-tile edge-chunk counts, so the shared SPMD program's
per-position chunk count F[pos] (maxed over cores) wastes almost nothing.

Per tile, layer 1 splits into
  self-loop part: one fp8 DoubleRow matmul from a dense preloaded block
      xselfT[feat, node] * dinv2  ->  psum_h1 (start)
  edge part: host-staged per-core edge streams (gathered source rows
      pre-multiplied by norm, fp8, partition-major so every DMA reads
      >=2KB contiguous per partition). Per chunk:
      x_chunk fp8 DoubleRow (K=256 in one op) -> psum_A -> cast fp8
      psum_h1 += Q^T @ xw  (one-hot Q streamed in the same blob)
Layer 2 + pooling collapse into G += [h1 pair]^T (x) [M pair] fp8
DoubleRow pairs accumulated in one PSUM bank; G is the only collective
(64KB AllReduce), then the tiny MLP head runs replicated on every core.

fp8 scale trick: W1, b1 are staged x16 (relu is positively homogeneous),
W2 staged /16 — keeps W1 out of fp8's subnormal range.
"""
import numpy as np

N_CORES = 8
N = 100000
NV = N + 1
S = 12544                 # nodes per core = 98 * 128
T_TILES = S // 128        # 98
G_TILES = N_CORES * T_TILES   # 784 global tiles
NVP = N_CORES * S
B = 128
IN = 256
HID = 128

W1_SCALE = 16.0
DEBUG = False


# ----------------------------------------------------------------- host prep
def _build_branch(s_e, d_e, batch):
    """s_e/d_e: edge endpoints WITHOUT self loops (E real + B virtual edges).

    Returns per-branch staging data. Self loops enter deg, M and the dense
    per-tile self blocks, not the edge stream.
    """
    deg = np.bincount(d_e, minlength=NV).astype(np.float64) + 1.0
    dinv = 1.0 / np.sqrt(deg)
    enorm = (dinv[s_e] * dinv[d_e]).astype(np.float64)

    # M = T @ A over full A (edges + self loops)
    es = np.concatenate([s_e, np.arange(NV, dtype=np.int64)])
    ed = np.concatenate([d_e, np.arange(NV, dtype=np.int64)])
    en = np.concatenate([enorm, 1.0 / deg])
    M = np.zeros((B, NVP), dtype=np.float64)
    real = ed < N
    np.add.at(M, (batch[ed[real]].astype(np.int64), es[real]), en[real])
    virt = ~real
    if virt.any():
        M += np.bincount(es[virt], weights=en[virt], minlength=NVP)[None, :]

    # balanced tile -> (core, pos) assignment over edge-chunk counts
    gt = d_e // 128
    counts = np.bincount(gt, minlength=G_TILES)
    ct = -(-counts // 128)                      # ceil, 0 for empty tiles
    order = np.argsort(-ct, kind="stable")      # tiles by count desc
    tile_at = np.empty((N_CORES, T_TILES), dtype=np.int64)
    for r, tl in enumerate(order):
        row, idx = divmod(r, N_CORES)
        core = idx if (row % 2 == 0) else N_CORES - 1 - idx
        tile_at[core, row] = tl
    F = ct[order[::N_CORES]].astype(np.int64)   # per-position max over cores
    C = int(F.sum())
    chunk_base = np.concatenate([[0], np.cumsum(F)])

    # group edge entries by global tile
    eorder = np.argsort(gt, kind="stable")
    es_s, ed_s, en_s = s_e[eorder], d_e[eorder], enorm[eorder]
    tile_starts = np.concatenate([[0], np.cumsum(counts)])

    ent_src = np.zeros((N_CORES, C * 128), dtype=np.int64)
    ent_norm = np.zeros((N_CORES, C * 128), dtype=np.float32)
    ent_slot = np.zeros((N_CORES, C * 128), dtype=np.int64)
    for k in range(N_CORES):
        for t in range(T_TILES):
            tl = tile_at[k, t]
            a, bnd = tile_starts[tl], tile_starts[tl + 1]
            if bnd == a:
                continue
            off = chunk_base[t] * 128
            m = bnd - a
            ent_src[k, off:off + m] = es_s[a:bnd]
            ent_norm[k, off:off + m] = en_s[a:bnd]
            ent_slot[k, off:off + m] = ed_s[a:bnd] - tl * 128
    return dict(ent_src=ent_src, ent_norm=ent_norm, ent_slot=ent_slot,
                F=F, C=C, M=M, tile_at=tile_at, dinv2=(1.0 / deg))


def _host_prep(x, emb_w, edge_index, batch):
    xv = np.concatenate([np.asarray(x, np.float32),
                         np.asarray(emb_w, np.float32)], axis=0)
    roots = np.searchsorted(batch, np.arange(B, dtype=batch.dtype)).astype(np.int64)
    ei0 = edge_index[0].astype(np.int64)
    ei1 = edge_index[1].astype(np.int64)
    vs = np.full(B, N, dtype=np.int64)
    br = {
        "td": _build_branch(np.concatenate([ei0, vs]), np.concatenate([ei1, roots]), batch),
        "bu": _build_branch(np.concatenate([ei1, roots]), np.concatenate([ei0, vs]), batch),
    }
    counts_g = np.bincount(batch, minlength=B).astype(np.float64)
    return xv, br, counts_g


# ------------------------------------------------------- walrus wait limiter
def _split_excess_waits(nc, limit=1):
    import concourse.mybir as mybir
    n_added = 0
    for bb in nc.main_func.blocks:
        insts = bb.instructions
        new_list = []
        for inst in insts:
            si = inst.sync_info
            if si is not None and si.on_wait and len(si.on_wait) > limit:
                waits = list(si.on_wait)
                extra, keep = waits[:-limit], waits[-limit:]
                for w in extra:
                    noop = mybir.InstNoOp(name=f"I-wsplit-{nc.next_id()}", ins=[], outs=[])
                    noop.engine = inst.engine
                    noop.sync_info = mybir.SyncInfo(on_wait=[w], on_update=[])
                    nc.register_instruction(noop, overwrite=True)
                    new_list.append(noop)
                    n_added += 1
                inst.sync_info = mybir.SyncInfo(on_wait=keep, on_update=list(si.on_update or []))
            new_list.append(inst)
        insts[:] = new_list
    return n_added


# ------------------------------------------------------------ device program
def _build_program(F_td, F_bu):
    import concourse.bass as bass
    import concourse.mybir as mybir
    import concourse.tile as tile

    f32 = mybir.dt.float32
    bf16 = mybir.dt.bfloat16
    fp8 = mybir.dt.float8e4
    DR = mybir.MatmulPerfMode.DoubleRow

    nc = bass.Bass(target_bir_lowering=False, trn_type="TRN2", num_swdge_queues=4)

    dram_in = {}
    for bn, C in (("td", int(F_td.sum())), ("bu", int(F_bu.sum()))):
        dram_in[f"xs_{bn}"] = nc.dram_tensor(f"xs_{bn}", [128, C * 3, 128], fp8, kind="ExternalInput")
        dram_in[f"xself_{bn}"] = nc.dram_tensor(f"xself_{bn}", [128, T_TILES * 2, 128], fp8, kind="ExternalInput")
        dram_in[f"mt_{bn}"] = nc.dram_tensor(f"mt_{bn}", [128, T_TILES, 128], fp8, kind="ExternalInput")
        dram_in[f"w1_{bn}"] = nc.dram_tensor(f"w1_{bn}", [128, 2, HID], fp8, kind="ExternalInput")
        dram_in[f"b1b_{bn}"] = nc.dram_tensor(f"b1b_{bn}", [128, 4, HID], f32, kind="ExternalInput")
        dram_in[f"w2_{bn}"] = nc.dram_tensor(f"w2_{bn}", [HID, HID], bf16, kind="ExternalInput")
        dram_in[f"pb_{bn}"] = nc.dram_tensor(f"pb_{bn}", [HID, B], f32, kind="ExternalInput")
    dram_in["pw1"] = nc.dram_tensor("pw1", [2 * HID, 2 * HID], f32, kind="ExternalInput")
    dram_in["pb1"] = nc.dram_tensor("pb1", [128, 2], f32, kind="ExternalInput")
    dram_in["pw2"] = nc.dram_tensor("pw2", [2 * HID, HID], f32, kind="ExternalInput")
    dram_in["pb2"] = nc.dram_tensor("pb2", [128, 1], f32, kind="ExternalInput")
    out_t = nc.dram_tensor("outT", [HID, B], f32, kind="ExternalOutput")

    N_GRP = (T_TILES + 3) // 4

    with tile.TileContext(nc) as tc:
        with (
            tc.tile_pool(name="const", bufs=1) as cpool,
            tc.tile_pool(name="stream", bufs=6) as spool,
            tc.tile_pool(name="selfp", bufs=3) as selfpool,
            tc.tile_pool(name="mtp", bufs=3) as mtpool,
            tc.tile_pool(name="work", bufs=8) as wpool,
            tc.tile_pool(name="psA", bufs=3, space="PSUM") as psA,
            tc.tile_pool(name="psH", bufs=3, space="PSUM") as psH,
            tc.tile_pool(name="psG", bufs=2, space="PSUM") as psG,
            tc.tile_pool(name="dram", bufs=1, space="DRAM") as dpool,
        ):
            dma_engines = [nc.sync, nc.scalar, nc.gpsimd]
            dma_rr = [0]

            def next_eng():
                eng = dma_engines[dma_rr[0] % len(dma_engines)]
                dma_rr[0] += 1
                return eng

            # ---- per-branch small constants -----------------------------
            consts = {}
            for bn in ("td", "bu"):
                w1sb = cpool.tile([128, 2, HID], fp8, name=f"w1sb_{bn}")
                nc.sync.dma_start(w1sb[:], dram_in[f"w1_{bn}"][:, :, :])
                b1b4 = cpool.tile([128, 4, HID], f32, name=f"b1b4_{bn}")
                nc.scalar.dma_start(b1b4[:], dram_in[f"b1b_{bn}"][:, :, :])
                consts[bn] = (w1sb, b1b4)

            ar_out = {}
            for bn, F in (("td", F_td), ("bu", F_bu)):
                C = int(F.sum())
                xs = dram_in[f"xs_{bn}"]
                xself_d = dram_in[f"xself_{bn}"]
                mt_d = dram_in[f"mt_{bn}"]
                w1sb, b1b4 = consts[bn]

                # just-in-time per-group xself/mt slices (1 group lookahead)
                xself_t = {}
                mt_t = {}

                def load_group(g):
                    if g >= N_GRP or g in xself_t:
                        return
                    ns_g = min(4, T_TILES - g * 4)
                    xt = selfpool.tile([128, 8, 128], fp8, name="xselfg")
                    next_eng().dma_start(xt[:, 0:2 * ns_g, :],
                                         xself_d[:, g * 8:g * 8 + 2 * ns_g, :])
                    mtt = mtpool.tile([128, 4, 128], fp8, name="mtg")
                    next_eng().dma_start(mtt[:, 0:ns_g, :],
                                         mt_d[:, g * 4:g * 4 + ns_g, :])
                    xself_t[g] = xt
                    mt_t[g] = mtt

                load_group(0)
                load_group(1)

                psum_G = psG.tile([HID, B], f32, name=f"psum_G_{bn}", tag="G")

                xws_grp = None
                psum_A = None
                cast_rr = [0]

                def emit_qmms(lst):
                    # merge eligible (same tile, adjacent chunk) pairs into
                    # one fp8 DoubleRow Q-matmul (strided lhsT view)
                    k = 0
                    while k < len(lst):
                        (xt, sl, xg, cc2, ph, tt2, sp) = lst[k]
                        if k + 1 < len(lst):
                            (xt_n, sl_n, xg_n, cc_n, ph_n, tt_n, sp_n) = lst[k + 1]
                            if (xt_n is xt and sl_n == sl + 3 and xg_n is xg
                                    and cc_n == cc2 + 1 and ph_n is ph
                                    and tt_n == tt2):
                                nc.tensor.matmul(ph[:, tt2, :],
                                                 xt[:, sl + 2:sl + 6:3, :],
                                                 xg[:, cc2:cc2 + 2, :],
                                                 start=False, stop=sp_n,
                                                 perf_mode=DR)
                                k += 2
                                continue
                        nc.tensor.matmul(ph[:, tt2, :], xt[:, sl + 2, :],
                                         xg[:, cc2, :], start=False, stop=sp)
                        k += 1

                def flush_pend(nslices, drain=False):
                    # cast current group (alternating DVE/ACT so neither
                    # saturates), emit the group-before-previous Q-matmuls
                    # (2-group software pipeline so PE never waits on casts)
                    if pend:
                        if cast_rr[0] % 2 == 0:
                            nc.vector.tensor_copy(xws_grp[:, 0:nslices, :],
                                                  psum_A[:, 0:nslices, :])
                        else:
                            nc.scalar.activation(xws_grp[:, 0:nslices, :],
                                                 psum_A[:, 0:nslices, :],
                                                 mybir.ActivationFunctionType.Copy)
                        cast_rr[0] += 1
                        emit_qmms(pend_prev2)
                        pend_prev2[:] = list(pend_prev)
                        pend_prev[:] = list(pend)
                        pend.clear()
                    if drain:
                        emit_qmms(pend_prev2 + pend_prev)
                        pend_prev2.clear()
                        pend_prev.clear()

                c = 0
                h1_grp = None
                psum_h1 = None
                xt2 = None
                pend: list = []
                pend_prev: list = []
                pend_prev2: list = []
                pend_m: list = []
                for t in range(T_TILES):
                    tt = t % 4
                    gi = t // 4
                    if tt == 0:
                        psum_h1 = psH.tile([128, 4, HID], f32, name="psum_h1", tag="H")
                        h1_grp = wpool.tile([128, 4, HID], fp8, name="h1_grp")
                        load_group(gi + 1)
                    ft = int(F[t])
                    xself = xself_t[gi]
                    # self-loop part: one DoubleRow, first write of this tile's
                    # psum slice. start=True only on the group's first matmul:
                    # start marks the WHOLE 2KB psum bank pending-zero, so a
                    # per-tile start would wipe earlier tiles whose pipelined
                    # Q-matmuls land later. Pending-zero propagation makes the
                    # first write to each slice an overwrite either way.
                    nc.tensor.matmul(psum_h1[:, tt, :], xself[:, 2 * tt:2 * tt + 2, :],
                                     w1sb[:, :, :], start=(tt == 0), stop=(ft == 0),
                                     perf_mode=DR)
                    for j in range(ft):
                        cc = c % 4
                        if cc == 0:
                            psum_A = psA.tile([128, 4, HID], f32, name="psum_A", tag="A")
                            xws_grp = wpool.tile([128, 4, HID], fp8, name="xws_grp")
                        if c % 8 == 0:
                            nld = min(8, C - c)
                            xt2 = spool.tile([128, 24, 128], fp8, name="xt2")
                            next_eng().dma_start(xt2[:, 0:nld * 3, :],
                                                 xs[:, c * 3:(c + nld) * 3, :])
                        sl = (c % 8) * 3
                        nc.tensor.matmul(psum_A[:, cc, :], xt2[:, sl:sl + 2, :],
                                         w1sb[:, :, :], start=True, stop=True,
                                         perf_mode=DR)
                        pend.append((xt2, sl, xws_grp, cc, psum_h1, tt,
                                     j == ft - 1))
                        if cc == 3 or c == C - 1:
                            flush_pend(cc + 1)
                        c += 1
                    if tt == 3 or t == T_TILES - 1:
                        flush_pend(((c - 1) % 4) + 1 if pend else 0, drain=True)
                        ns = tt + 1
                        tmp = wpool.tile([128, 4, HID], f32, name="h1tmp")
                        nc.vector.tensor_tensor(tmp[:, 0:ns, :], psum_h1[:, 0:ns, :],
                                                b1b4[:, 0:ns, :], op=mybir.AluOpType.add)
                        nc.scalar.activation(h1_grp[:, 0:ns, :], tmp[:, 0:ns, :],
                                             mybir.ActivationFunctionType.Relu)
                        t0 = t - tt
                        # G += h1-pair (x) M-pair, one DoubleRow per 2 tiles,
                        # one 4-tile group delayed so PE never waits on relu
                        for (h1p, pgi, pns) in pend_m:
                            for jj in range(0, pns, 2):
                                nc.tensor.matmul(
                                    psum_G[:], h1p[:, jj:jj + 2, :],
                                    mt_t[pgi][:, jj:jj + 2, :],
                                    start=(pgi == 0 and jj == 0), stop=False,
                                    perf_mode=DR)
                        pend_m = [(h1_grp, gi, ns)]
                        if t == T_TILES - 1:
                            for (h1p, pgi, pns) in pend_m:
                                for jj in range(0, pns, 2):
                                    nc.tensor.matmul(
                                        psum_G[:], h1p[:, jj:jj + 2, :],
                                        mt_t[pgi][:, jj:jj + 2, :],
                                        start=(pgi == 0 and jj == 0),
                                        stop=(pgi * 4 + jj + 2 >= T_TILES),
                                        perf_mode=DR)
                            pend_m = []
                # per-branch bf16 AllReduce: td's overlaps bu compute
                g = cpool.tile([HID, B], bf16, name=f"g_{bn}")
                nc.vector.tensor_copy(g[:], psum_G[:])
                arin = dpool.tile([HID, B], bf16, name=f"arin_{bn}")
                arout = dpool.tile([HID, B], bf16, addr_space="Shared", name=f"arout_{bn}")
                nc.gpsimd.dma_start(arin[:], g[:])
                nc.gpsimd.collective_compute(
                    "AllReduce", mybir.AluOpType.add,
                    replica_groups=[list(range(N_CORES))],
                    ins=[arin[:]], outs=[arout[:]],
                )
                ar_out[bn] = arout
                if bn == "td":
                    # head weights: prefetch mid-program, off the ramp path
                    pw1 = cpool.tile([128, 2, 2 * HID], f32)
                    nc.gpsimd.dma_start(pw1[:], dram_in["pw1"].rearrange("(kc p) n -> p kc n", p=128))
                    pb1 = cpool.tile([128, 2], f32)
                    nc.gpsimd.dma_start(pb1[:], dram_in["pb1"][:, :])
                    pw2 = cpool.tile([128, 2, HID], f32)
                    nc.gpsimd.dma_start(pw2[:], dram_in["pw2"].rearrange("(kc p) n -> p kc n", p=128))
                    pb2 = cpool.tile([128, 1], f32)
                    nc.gpsimd.dma_start(pb2[:], dram_in["pb2"][:, :])
                    w2sb = {}
                    pbsb = {}
                    for bn2 in ("td", "bu"):
                        w2sb[bn2] = cpool.tile([HID, HID], bf16, name=f"w2sb_{bn2}")
                        nc.sync.dma_start(w2sb[bn2][:], dram_in[f"w2_{bn2}"][:, :])
                        pbsb[bn2] = cpool.tile([HID, B], f32, name=f"pbsb_{bn2}")
                        nc.scalar.dma_start(pbsb[bn2][:], dram_in[f"pb_{bn2}"][:, :])

            # ---- MLP head (replicated on every core, transposed layout) ----
            pool_t = {}
            for i, bn in enumerate(("td", "bu")):
                garr = cpool.tile([HID, B], bf16, name=f"garr_{bn}")
                nc.gpsimd.dma_start(garr[:], ar_out[bn][:])
                ps_p = psA.tile([HID, B], f32, name="ps_p", tag="A")
                nc.tensor.matmul(ps_p[:], w2sb[bn][:], garr[:],
                                 start=True, stop=True)
                pt = cpool.tile([HID, B], f32, name=f"pool_{bn}")
                nc.vector.tensor_tensor(pt[:], ps_p[:], pbsb[bn][:], op=mybir.AluOpType.add)
                pool_t[bn] = pt                                      # pooled^T [f, g]

            r1 = []
            for hh in range(2):
                ps1 = psA.tile([128, B], f32, name="ps1", tag="A")
                nc.tensor.matmul(ps1[:], pw1[:, 0, hh * 128:(hh + 1) * 128],
                                 pool_t["bu"][:], start=True, stop=False)
                nc.tensor.matmul(ps1[:], pw1[:, 1, hh * 128:(hh + 1) * 128],
                                 pool_t["td"][:], start=False, stop=True)
                r = wpool.tile([128, B], f32, name=f"r1_{hh}")
                nc.scalar.activation(r[:], ps1[:], mybir.ActivationFunctionType.Relu,
                                     bias=pb1[:, hh:hh + 1])
                r1.append(r)
            ps2 = psH.tile([HID, B], f32, name="ps2", tag="H")
            for hh in range(2):
                nc.tensor.matmul(ps2[:], pw2[:, hh, :], r1[hh][:],
                                 start=(hh == 0), stop=(hh == 1))
            ofin = wpool.tile([HID, B], f32, name="ofin")
            nc.vector.tensor_scalar(ofin[:], ps2[:], pb2[:, 0:1], None,
                                    op0=mybir.AluOpType.add)
            nc.gpsimd.dma_start(out_t[:, :], ofin[:])

    _split_excess_waits(nc, limit=1)
    return nc


# ------------------------------------------------------------------- staging
def _stage_core(k, xvp_f32, br, counts_g, inputs, np_fp8):
    m = {}
    for bn in ("td", "bu"):
        d = br[bn]
        C = d["C"]
        tiles = d["tile_at"][k]                         # [98] global tile ids

        # edge stream blob [128, C*3, 128]: per chunk 3 slices (x-k0, x-k1, Q)
        src = d["ent_src"][k]
        nrm = d["ent_norm"][k]
        xg = xvp_f32[src] * nrm[:, None]                # [C*128, 256] f32
        xpart = xg.reshape(C, 128, IN).transpose(2, 0, 1)   # [256, C, 128]
        xpart = xpart.reshape(2, 128, C, 128).transpose(1, 2, 0, 3)  # [128,C,2,128]
        slot = d["ent_slot"][k]
        Q = np.zeros((C, 128, 128), dtype=np.float32)
        Q.reshape(C * 128, 128)[np.arange(C * 128), slot] = 1.0
        qpart = Q.transpose(1, 0, 2)                    # [128, C, 128]
        blob = np.concatenate([xpart, qpart[:, :, None, :]], axis=2)  # [128,C,3,128]
        m[f"xs_{bn}"] = np.ascontiguousarray(
            blob.reshape(128, C * 3, 128), dtype=np_fp8)

        # dense self block [128, 98*2, 128]: xselfT[feat, tile, kc, node]
        nodes = (tiles[:, None] * 128 + np.arange(128)[None, :]).reshape(-1)
        xs_blk = xvp_f32[nodes] * d["dinv2"][np.minimum(nodes, NV - 1)][:, None]
        xs_blk[nodes >= NV] = 0.0
        A = xs_blk.reshape(T_TILES, 128, IN).transpose(2, 0, 1)  # [256, 98, 128]
        A = A.reshape(2, 128, T_TILES, 128).transpose(1, 2, 0, 3)  # [128,98,2,128]
        m[f"xself_{bn}"] = np.ascontiguousarray(
            A.reshape(128, T_TILES * 2, 128), dtype=np_fp8)

        # M^T columns for this core's tiles, [128, 98, 128] = [node, tile, graph]
        Mc = d["M"][:, nodes].reshape(B, T_TILES, 128)  # [g, t, p]
        m[f"mt_{bn}"] = np.ascontiguousarray(Mc.transpose(2, 1, 0), dtype=np_fp8)

        w1 = np.asarray(inputs[f"{bn}_w1"], np.float32) * W1_SCALE
        m[f"w1_{bn}"] = np.ascontiguousarray(
            w1.reshape(2, 128, HID).transpose(1, 0, 2), dtype=np_fp8)
        b1b = np.broadcast_to(
            np.asarray(inputs[f"{bn}_b1"], np.float32) * W1_SCALE, (128, 4, HID))
        m[f"b1b_{bn}"] = np.ascontiguousarray(b1b, dtype=np.float32)
        import ml_dtypes as _mld
        m[f"w2_{bn}"] = np.ascontiguousarray(
            np.asarray(inputs[f"{bn}_w2"], np.float32) / W1_SCALE,
            dtype=_mld.bfloat16)
        m[f"pb_{bn}"] = np.ascontiguousarray(
            np.outer(np.asarray(inputs[f"{bn}_b2"], np.float64), counts_g + 1.0),
            dtype=np.float32)
    m["pw1"] = np.ascontiguousarray(np.asarray(inputs["p_w1"], np.float32))
    m["pb1"] = np.ascontiguousarray(
        np.asarray(inputs["p_b1"], np.float32).reshape(2, 128).T)
    m["pw2"] = np.ascontiguousarray(np.asarray(inputs["p_w2"], np.float32))
    m["pb2"] = np.asarray(inputs["p_b2"], np.float32).reshape(128, 1).copy()
    return m


def _enable_ldw_opt():
    import os, stat, tempfile
    from concourse import bass_utils
    if getattr(bass_utils, "_ldw_shim", None):
        return
    real = bass_utils.get_walrus_driver()
    shim = os.path.join(tempfile.gettempdir(), "walrus_ldw_shim.sh")
    with open(shim, "w") as f:
        f.write("#!/bin/sh\nargs=\"\"\nfor a in \"$@\"; do\n"
                "  case \"$a\" in --enable-ldw-opt=false) a=--enable-ldw-opt=true;; esac\n"
                "  args=\"$args $a\"\ndone\nexec %s $args\n" % real)
    os.chmod(shim, stat.S_IRWXU)
    bass_utils.get_walrus_driver = lambda: shim
    bass_utils._ldw_shim = shim


def _run(inputs, trace=False):
    import ml_dtypes
    from concourse import bass_utils
    # NOTE: walrus --enable-ldw-opt=true rejects DoubleRow ldweights
    # ("InstLdweights is not compatible with LDW optimization") — keep off.

    x = np.asarray(inputs["x"])
    edge_index = np.asarray(inputs["edge_index"])
    batch = np.asarray(inputs["batch"])
    xv, br, counts_g = _host_prep(x, inputs["emb_w"], edge_index, batch)
    xvp = np.zeros((NVP, IN), dtype=np.float32)
    xvp[:NV] = xv

    np_fp8 = ml_dtypes.float8_e4m3
    in_maps = [_stage_core(k, xvp, br, counts_g, inputs, np_fp8)
               for k in range(N_CORES)]
    nc = _build_program(br["td"]["F"], br["bu"]["F"])
    last = None
    for attempt in range(3):
        try:
            res = bass_utils.run_bass_kernel_spmd(
                nc, in_maps, core_ids=list(range(N_CORES)), trace=trace)
            break
        except Exception as e:   # transient NRT device errors recover on retry
            last = e
    else:
        raise last
    out = np.ascontiguousarray(res.results[0]["outT"].T, dtype=np.float32)
    return out, res


def kernel(**inputs) -> np.ndarray:
    out, _ = _run(inputs, trace=False)
    return out
